# revision 33
# baseline (speedup 1.0000x reference)
# AdaOctConv distributed Trainium2 kernel (8 NeuronCores, SPMD, no collectives).
#
# Sharding: 4 samples x 2 spatial halves = 8 cores. Each core computes a
# 64-row band of hf_out and a 32-row band of lf_out of one sample. Internal
# split halos come in via host-side overlapped shards; image-border halos are
# reflect-copies of computed interior rows, applied on device as mask-blended
# row/col fixups so all 8 cores share one SPMD graph.
#
# Math folding (per channel c, per sample):
#   instance_norm -> depthwise3x3(per-sample w) -> *w_point + bias  ==
#   depthwise3x3(raw x; ws*(wp*rsqrt(var+eps))) + beta, with beta folded into
#   the following dense conv's bias. The heavy work is six 3x3 dense convs,
#   each computed as 9 shift-matmuls (bf16 in, fp32 PSUM accumulation).
import sys

for _p in ("/opt/trn_rl_repo",):
    if _p not in sys.path:
        sys.path.append(_p)

import os

import numpy as np

B, C = 4, 256
EPS, SLOPE = 1e-5, 0.01
NCORES = 8

XR, XC = 72, 136      # x band (reflect-padded +-4)
ZR, ZC = 70, 134      # depthwise output band
HR, HC = 68, 132      # hf band
OR_, OC_ = 64, 128    # hf output band
XLR, XLC = 38, 70
ZLR, ZLC = 36, 68
LR, LC = 34, 66
OLR, OLC = 32, 64
UR, UC = 66, 130      # upsampled-lf band
UPT = 10              # rows per up tile (8 tiles of [8g, 8g+10))

_CACHE = {}


def _split_multi_waits(nc, mybir):
    """This walrus build accepts at most one sync-wait per instruction;
    hoist extra waits onto single-wait NOPs on the same engine."""
    for fn in nc.m.functions:
        for bb in fn.blocks:
            new = []
            for ins in bb.instructions:
                si = ins.sync_info
                if si and si.on_wait and len(si.on_wait) > 1:
                    waits = list(si.on_wait)
                    for w in waits[:-1]:
                        nop = mybir.InstNoOp(
                            name=f"I-ws-{nc.next_id()}", ins=[], outs=[]
                        )
                        nop.engine = ins.engine
                        nop.sync_info = mybir.SyncInfo(on_wait=[w], on_update=[])
                        new.append(nop)
                    ins.sync_info = mybir.SyncInfo(
                        on_wait=[waits[-1]], on_update=list(si.on_update or [])
                    )
                new.append(ins)
            bb.instructions[:] = new


def _build_nc():
    import contextlib

    import concourse.bass as bass
    import concourse.mybir as mybir
    import concourse.tile as tile

    f32 = mybir.dt.float32
    bf16 = mybir.dt.bfloat16
    AF = mybir.ActivationFunctionType
    OP = mybir.AluOpType
    AX = mybir.AxisListType

    STAGES = int(os.environ.get("ADAOCT_STAGES", "9"))
    SPLIT = os.environ.get("ADAOCT_SPLIT", "1") == "1"

    nc = bass.Bass()

    def par(name, shape, out=False):
        return nc.declare_dram_parameter(name, list(shape), f32, isOutput=out)

    xh = par("xh", (C, XR, XC))
    W1N = ("whpw", "whbw", "wlpw", "wlbw")
    BN = ("hsb", "hpb", "hbb", "ahb", "lsb", "lpb", "lbb", "alb")
    xh_ot = par("xh_ot", (C, 64, 128))
    xl = par("xl", (C, XLR, XLC))
    xl_ot = par("xl_ot", (C, 32, 64))
    identp = par("identp", (128, 128))
    maskp = par("maskp", (128, 1))
    wsets = {
        n: par(n, (C, 9, C))
        for n in ("whsw", "wlsw", "wadal", "wadah", "wh2h", "wl2h", "wl2l", "wh2l")
    }
    w1all = par("w1all", (C, 4 * C))  # whpw|whbw|wlpw|wlbw along cols
    ball = par("ball", (C, 8))  # hsb hpb hbb ahb lsb lpb lbb alb
    sall = par("sall", (C, 2, 5, 5))  # sh | sl
    out_hf = par("out_hf", (C, OR_, OC_), out=True)
    out_lf = par("out_lf", (C, OLR, OLC), out=True)

    TAPS = [(dy, dx) for dy in range(3) for dx in range(3)]

    class _StopBuild(Exception):
        pass

    with tile.TileContext(nc) as tc:
        es = contextlib.ExitStack()
        with es, contextlib.suppress(_StopBuild):
            persist = es.enter_context(tc.tile_pool(name="persist", bufs=1))
            small = es.enter_context(tc.tile_pool(name="small", bufs=1))
            wpool = es.enter_context(tc.tile_pool(name="wpool", bufs=4))
            wstg = es.enter_context(tc.tile_pool(name="wstg", bufs=2))
            psmall = es.enter_context(tc.tile_pool(name="psmall", bufs=2, space="PSUM"))
            mmp = es.enter_context(tc.tile_pool(name="mmp", bufs=6, space="PSUM"))
            accp = es.enter_context(tc.tile_pool(name="accp", bufs=1))
            z2p = es.enter_context(tc.tile_pool(name="z2p", bufs=4))
            upp = es.enter_context(tc.tile_pool(name="upp", bufs=2))
            outp = es.enter_context(tc.tile_pool(name="outp", bufs=2))
            fixp = es.enter_context(tc.tile_pool(name="fixp", bufs=1))

            # ---------- persistent tensors ----------
            xh_sb = [persist.tile([128, XR, XC], bf16, tag=f"xh{i}") for i in range(2)]
            xl_sb = [persist.tile([128, XLR, XLC], bf16, tag=f"xl{i}") for i in range(2)]
            hf_sb = [persist.tile([128, HR, HC], bf16, tag=f"hf{i}") for i in range(2)]
            lf_sb = [persist.tile([128, LR, LC], bf16, tag=f"lf{i}") for i in range(2)]
            avg_sb = [persist.tile([128, LR, LC], bf16, tag=f"av{i}") for i in range(2)]

            eps_t = small.tile([128, 1], f32, tag="eps")
            nc.vector.memset(eps_t, EPS)
            mask_t = small.tile([128, 1], f32, tag="mask")
            nc.sync.dma_start(out=mask_t, in_=maskp[:, :])
            imask_t = small.tile([128, 1], f32, tag="imask")
            nc.scalar.activation(
                out=imask_t, in_=mask_t, func=AF.Copy, bias=1.0, scale=-1.0
            )

            identf = small.tile([128, 128], f32, tag="identf", name="identf")
            nc.sync.dma_start(out=identf, in_=identp[:, :])
            ident_b = small.tile([128, 128], bf16, tag="identb", name="identb")
            nc.scalar.copy(out=ident_b, in_=identf)

            def blend(cur, ref, sel, cols):
                """cur = cur + sel*(ref - cur) over a [128, 1, cols] slice."""
                d = fixp.tile([128, 1, 144], f32, tag="fixd")
                dd = d[:, :, :cols]
                nc.vector.tensor_sub(out=dd, in0=ref, in1=cur)
                nc.vector.scalar_tensor_tensor(
                    out=cur, in0=dd, scalar=sel, in1=cur, op0=OP.mult, op1=OP.add
                )

            bias_sb = {}
            for i in range(2):
                bt = small.tile([128, 8], f32, tag=f"ball{i}", name=f"ball{i}")
                nc.sync.dma_start(out=bt, in_=ball[i * 128 : (i + 1) * 128, :])
                for j, n in enumerate(BN):
                    bias_sb[(n, i)] = bt[:, j : j + 1]

            w1_sb = {}
            for i in range(2):
                stg = wstg.tile([128, 1152], f32, tag="wstg", name="w1stg")
                nc.sync.dma_start(
                    out=stg[:, : 4 * C].rearrange("c (k o) -> c k o", k=4),
                    in_=w1all[i * 128 : (i + 1) * 128, :].rearrange(
                        "c (k o) -> c k o", k=4),
                )
                t = small.tile([128, 4, C], bf16, tag=f"w1_{i}", name=f"w1_{i}")
                nc.scalar.copy(out=t, in_=stg[:, : 4 * C].rearrange(
                    "c (k o) -> c k o", k=4))
                for j, n in enumerate(W1N):
                    w1_sb[(n, i)] = t[:, j, :]

            sbuf_s = {}
            for i in range(2):
                tf = small.tile([128, 2, 5, 5], f32, tag=f"sf{i}", name=f"sf{i}")
                nc.sync.dma_start(out=tf, in_=sall[i * 128 : (i + 1) * 128, :, :, :])
                tb = small.tile([128, 2, 5, 5], bf16, tag=f"sb{i}", name=f"sb{i}")
                nc.scalar.copy(out=tb, in_=tf)
                for j, nm in enumerate(("sh", "sl")):
                    sbuf_s[(nm, i, "f")] = tf[:, j]
                    sbuf_s[(nm, i, "b")] = tb[:, j]

            # ---------- weight set loading ----------
            wcache = {}

            def wset(name):
                if name in wcache:
                    return wcache[name]
                p = wsets[name]
                tiles = []
                for i in range(2):
                    t = wpool.tile([128, 9, 2, 128], bf16, tag="wset")
                    flat = t.rearrange("c t o q -> c (t o q)")
                    src = p[i * 128 : (i + 1) * 128, :, :].rearrange("c t o -> c (t o)")
                    for j in range(2):
                        stg = wstg.tile([128, 1152], f32, tag="wstg")
                        nc.sync.dma_start(
                            out=stg, in_=src[:, j * 1152 : (j + 1) * 1152]
                        )
                        nc.scalar.copy(out=flat[:, j * 1152 : (j + 1) * 1152], in_=stg)
                    tiles.append(t)
                wcache[name] = tiles
                return tiles

            # ---------- instance-norm stats ----------
            # mean/var over [own band interior (bf16, in SBUF)] + [other
            # half's rows, streamed f32]. Sums via ScalarE accum_out.
            def stats_stream(other, hw, bpool):
                npc = (hw // 2) // 1024
                bn_t = []
                for i in range(2):
                    nm = f"{other.name}{i}"
                    bnst = small.tile([128, 2 * npc, 6], f32, tag=f"bn_{nm}",
                                      name=f"bn_{nm}")
                    flat = other[i * 128 : (i + 1) * 128, :, :].rearrange(
                        "c h w -> c (h w)"
                    )
                    for k in range(npc):
                        pc = bpool.tile([128, 1024], f32, tag="xstg2", name="xstg2")
                        nc.gpsimd.dma_start(
                            out=pc, in_=flat[:, k * 1024 : (k + 1) * 1024]
                        )
                        for j in range(2):
                            nc.vector.bn_stats(
                                out=bnst[:, 2 * k + j, :],
                                in_=pc[:, j * 512 : (j + 1) * 512],
                            )
                    bn_t.append(bnst)
                return bn_t

            def stats(bn_t, band_sb, bint, hw, bpool):
                r0, r1, c0, c1 = bint
                half = hw // 2
                m_t, inv_t = [], []
                for i in range(2):
                    nm = bn_t[i].tensor.name
                    sums = small.tile([128, 5, 2], f32, tag=f"s5_{nm}",
                                      name=f"s5_{nm}")
                    mvs = small.tile([128, 2], f32, tag=f"mvs_{nm}",
                                     name=f"mvs_{nm}")
                    nc.vector.bn_aggr(out=mvs, in_=bn_t[i])
                    # stream half back to (sum, sumsq) space
                    nc.vector.tensor_scalar_mul(out=sums[:, 4, 0:1],
                                                in0=mvs[:, 0:1], scalar1=float(half))
                    msq = small.tile([128, 1], f32, tag=f"msq_{nm}",
                                     name=f"msq_{nm}")
                    nc.vector.tensor_mul(out=msq, in0=mvs[:, 0:1], in1=mvs[:, 0:1])
                    nc.vector.tensor_add(out=msq, in0=msq, in1=mvs[:, 1:2])
                    nc.vector.tensor_scalar_mul(out=sums[:, 4, 1:2], in0=msq,
                                                scalar1=float(half))
                    rows = r1 - r0
                    q = rows // 4
                    for j in range(4):
                        seg = band_sb[i][:, r0 + j * q : r0 + (j + 1) * q, c0:c1]
                        scr = bpool.tile([128, 2048], bf16, tag="scr", name="scr",
                                         bufs=1)
                        sg = scr[:, : seg.free_size()]
                        nc.scalar.activation(out=sg, in_=seg, func=AF.Copy,
                                             accum_out=sums[:, j, 0:1])
                        nc.scalar.activation(out=sg, in_=seg, func=AF.Square,
                                             accum_out=sums[:, j, 1:2])
                    mv = small.tile([128, 2], f32, tag=f"mv_{nm}", name=f"mv_{nm}")
                    nc.vector.tensor_reduce(
                        out=mv, in_=sums.rearrange("c n k -> c k n"),
                        axis=AX.X, op=OP.add)
                    m = small.tile([128, 1], f32, tag=f"m_{nm}", name=f"m_{nm}")
                    nc.vector.tensor_scalar_mul(out=m, in0=mv[:, 0:1],
                                                scalar1=1.0 / hw)
                    var = small.tile([128, 1], f32, tag=f"v_{nm}", name=f"v_{nm}")
                    nc.vector.tensor_scalar_mul(out=var, in0=mv[:, 1:2],
                                                scalar1=1.0 / hw)
                    msqt = small.tile([128, 1], f32, tag=f"mq_{nm}", name=f"mq_{nm}")
                    nc.vector.tensor_mul(out=msqt, in0=m, in1=m)
                    nc.vector.tensor_sub(out=var, in0=var, in1=msqt)
                    inv = small.tile([128, 1], f32, tag=f"i_{nm}", name=f"i_{nm}")
                    nc.scalar.activation(
                        out=inv, in_=var, func=AF.Sqrt, bias=eps_t, scale=1.0
                    )
                    nc.vector.reciprocal(out=inv, in_=inv)
                    m_t.append(m)
                    inv_t.append(inv)
                return m_t, inv_t

            def load_band(param, dst, rows, cols, bpool, rchunk):
                for i in range(2):
                    src = param[i * 128 : (i + 1) * 128, :, :]
                    for r0 in range(0, rows, rchunk):
                        r1 = min(r0 + rchunk, rows)
                        stg = bpool.tile([128, 3264], f32, tag="xstg")
                        s3 = stg[:, : (r1 - r0) * cols].rearrange(
                            "c (r w) -> c r w", w=cols
                        )
                        nc.gpsimd.dma_start(out=s3, in_=src[:, r0:r1, :])
                        nc.scalar.copy(out=dst[i][:, r0:r1, :], in_=s3)

            bpool = es.enter_context(tc.tile_pool(name="bpool", bufs=2))
            wset("wlsw")
            wset("wadal")
            sums_l = stats_stream(xl_ot, 64 * 64, bpool)
            load_band(xl, xl_sb, XLR, XLC, bpool, 14)
            m_l, inv_l = stats(sums_l, xl_sb, (3, 35, 3, 67), 64 * 64, bpool)

            # ---------- kernel_predict + folds ----------
            # ws2 = ws*wp (stats-free; inv is folded into the dense-conv
            # stationary later). beta = b - (wp*inv)*m*S.
            def kp_A(skey, wsw_name, w1pw, w1bw, bsb, bpb, bbb, tag, want_diag=False):
                wsw = wset(wsw_name)
                s_f = [sbuf_s[(skey, i, "f")] for i in range(2)]
                s_b = [sbuf_s[(skey, i, "b")] for i in range(2)]
                pooled_b = []
                for i in range(2):
                    pf = small.tile([128, 1], f32, tag=f"poo{tag}{i}", name=f"poo{tag}{i}")
                    nc.vector.reduce_sum(out=pf, in_=s_f[i][:, 1:4, 1:4], axis=AX.XY)
                    pb = small.tile([128, 1], bf16, tag=f"poob{tag}{i}", name=f"poob{tag}{i}")
                    nc.scalar.copy(out=pb, in_=pf)
                    pooled_b.append(pb)

                ws2, wp_t, bv_t, S_t, diag = [], [], [], [], []
                for oc in range(2):
                    ps = psmall.tile([128, 9], f32, tag="ps9", name="ps9")
                    ps3 = ps.rearrange("c (a b) -> c a b", a=3)
                    n = 0
                    for t, (dy, dx) in enumerate(TAPS):
                        for ic in range(2):
                            nc.tensor.matmul(
                                ps3,
                                wsw[ic][:, t, oc, :],
                                s_b[ic][:, dy : dy + 3, dx : dx + 3],
                                start=(n == 0),
                                stop=(n == 17),
                            )
                            n += 1
                    wf = small.tile([128, 9], f32, tag=f"ws{tag}{oc}", name=f"ws{tag}{oc}")
                    nc.scalar.activation(out=wf, in_=ps, func=AF.Identity,
                                         bias=bias_sb[(bsb, oc)], scale=1.0)

                    ps1 = psmall.tile([128, 9], f32, tag="ps9", name="ps9")
                    for ic in range(2):
                        nc.tensor.matmul(
                            ps1[:, 0:1],
                            w1_sb[(w1pw, ic)][:, oc * 128 : (oc + 1) * 128],
                            pooled_b[ic],
                            start=(ic == 0),
                            stop=(ic == 1),
                        )
                    wp = small.tile([128, 1], f32, tag=f"wp{tag}{oc}", name=f"wp{tag}{oc}")
                    nc.scalar.activation(out=wp, in_=ps1[:, 0:1], func=AF.Identity,
                                         bias=bias_sb[(bpb, oc)], scale=1.0)
                    wp_t.append(wp)

                    ps2 = psmall.tile([128, 9], f32, tag="ps9", name="ps9")
                    for ic in range(2):
                        nc.tensor.matmul(
                            ps2[:, 0:1],
                            w1_sb[(w1bw, ic)][:, oc * 128 : (oc + 1) * 128],
                            pooled_b[ic],
                            start=(ic == 0),
                            stop=(ic == 1),
                        )
                    bv = small.tile([128, 1], f32, tag=f"bv{tag}{oc}", name=f"bv{tag}{oc}")
                    nc.scalar.activation(out=bv, in_=ps2[:, 0:1], func=AF.Identity,
                                         bias=bias_sb[(bbb, oc)], scale=1.0)
                    bv_t.append(bv)

                    w2 = small.tile([128, 9], f32, tag=f"w2{tag}{oc}", name=f"w2{tag}{oc}")
                    nc.scalar.activation(out=w2, in_=wf, func=AF.Identity,
                                         bias=0.0, scale=wp)
                    ws2.append(w2)
                    s_s = small.tile([128, 1], f32, tag=f"S{tag}{oc}", name=f"S{tag}{oc}")
                    nc.vector.reduce_sum(out=s_s, in_=wf, axis=AX.X)
                    S_t.append(s_s)
                    if want_diag:
                        dg = small.tile([128, 9, 128], bf16, tag=f"dg{tag}{oc}", name=f"dg{tag}{oc}")
                        for t in range(9):
                            nc.scalar.activation(
                                out=dg[:, t, :], in_=ident_b, func=AF.Identity,
                                bias=0.0, scale=w2[:, t : t + 1])
                        diag.append(dg)
                return ws2, diag, wp_t, bv_t, S_t

            def kp_B(wada_name, bada, wp_t, bv_t, S_t, m_t, inv_t, tag):
                """bias2 from unscaled wada, then scale wada in place by
                inv[cin] (per-partition)."""
                wada = wset(wada_name)
                beta_b = []
                for oc in range(2):
                    tmp = small.tile([128, 1], f32, tag=f"t1{tag}{oc}", name=f"t1{tag}{oc}")
                    nc.scalar.activation(out=tmp, in_=wp_t[oc], func=AF.Identity,
                                         bias=0.0, scale=inv_t[oc])
                    nc.scalar.activation(out=tmp, in_=tmp, func=AF.Identity,
                                         bias=0.0, scale=m_t[oc])
                    nc.scalar.activation(out=tmp, in_=tmp, func=AF.Identity,
                                         bias=0.0, scale=S_t[oc])
                    nc.scalar.mul(out=tmp, in_=tmp, mul=-1.0)
                    bb_ = small.tile([128, 1], bf16, tag=f"beb{tag}{oc}", name=f"beb{tag}{oc}")
                    nc.scalar.activation(out=bb_, in_=bv_t[oc], func=AF.Identity,
                                         bias=tmp, scale=1.0)
                    beta_b.append(bb_)
                bias2 = []
                for oc in range(2):
                    ps = psmall.tile([128, 9], f32, tag="ps9", name="ps9")
                    n = 0
                    for t in range(9):
                        for ic in range(2):
                            nc.tensor.matmul(
                                ps[:, 0:1],
                                wada[ic][:, t, oc, :],
                                beta_b[ic],
                                start=(n == 0),
                                stop=(n == 17),
                            )
                            n += 1
                    b2 = small.tile([128, 1], f32, tag=f"b2{tag}{oc}", name=f"b2{tag}{oc}")
                    nc.scalar.activation(out=b2, in_=ps[:, 0:1], func=AF.Identity,
                                         bias=bias_sb[(bada, oc)], scale=1.0)
                    bias2.append(b2)
                for ic in range(2):
                    wf_ = wada[ic].rearrange("c t o q -> c (t o q)")
                    nc.scalar.activation(out=wf_, in_=wf_, func=AF.Identity,
                                         bias=0.0, scale=inv_t[ic])
                return bias2, wada

            # ---------- depthwise (9-tap MAC on the Vector engine) ----------
            def dw_tile(x_sb, ws2, z0, zrows, W, tag, fix):
                """z[c, q, w] = sum_t ws2[c,t] * x[c, q+dy, w+dx]; cols
                [0, W-2) valid. Returns [128, zrows, W] bf16 views."""
                topq, tops, botq, bots, ctop, cts, cbot, cbs = fix
                zc = W - 2
                out = []
                for ic in range(2):
                    acc = accp.tile([128, 14, 134], bf16, tag="acc", name="acc")
                    a = acc[:, :zrows, :zc]
                    zt = z2p.tile([128, 14, zc], bf16, tag=f"z2{tag}", name=f"z2{tag}")
                    z = zt[:, :zrows, :]
                    zv = z
                    for t, (dy, dx) in enumerate(TAPS):
                        in0 = x_sb[ic][:, z0 + dy : z0 + dy + zrows, dx : dx + zc]
                        sc = ws2[ic][:, t : t + 1]
                        if t == 0:
                            nc.vector.tensor_scalar_mul(out=a, in0=in0, scalar1=sc)
                        elif t < 8:
                            nc.vector.scalar_tensor_tensor(
                                out=a, in0=in0, scalar=sc, in1=a,
                                op0=OP.mult, op1=OP.add,
                            )
                        else:
                            nc.vector.scalar_tensor_tensor(
                                out=zv, in0=in0, scalar=sc, in1=a,
                                op0=OP.mult, op1=OP.add,
                            )
                    if z0 <= topq < z0 + zrows:
                        blend(z[:, topq - z0 : topq - z0 + 1, :zc],
                              z[:, tops - z0 : tops - z0 + 1, :zc], imask_t, zc)
                    if z0 <= botq < z0 + zrows:
                        blend(z[:, botq - z0 : botq - z0 + 1, :zc],
                              z[:, bots - z0 : bots - z0 + 1, :zc], mask_t, zc)
                    nc.vector.tensor_copy(
                        out=z[:, :, ctop : ctop + 1], in_=z[:, :, cts : cts + 1]
                    )
                    nc.vector.tensor_copy(
                        out=z[:, :, cbot : cbot + 1], in_=z[:, :, cbs : cbs + 1]
                    )
                    out.append(z)
                return out

            def dw_tile_pe(x_sb, diag, z0, zrows, W, tag, fix):
                """Same contract as dw_tile, but computed on the TensorEngine
                as 9 accumulated diag-matmuls over flat 512-col windows."""
                topq, tops, botq, bots, ctop, cts, cbot, cbs = fix
                L = zrows * W - 2
                out = []
                for ic in range(2):
                    zt = z2p.tile([128, 14, W], bf16, tag=f"z2{tag}", name=f"z2p{tag}")
                    zflat = zt.rearrange("c r w -> c (r w)")
                    xflat = x_sb[ic].rearrange("c r w -> c (r w)")
                    for off in range(0, L, 512):
                        n = min(512, L - off)
                        ps = mmp.tile([128, 512], f32, tag="mm", name="zps")
                        for t, (dy, dx) in enumerate(TAPS):
                            base = (z0 + dy) * W + dx + off
                            nc.tensor.matmul(
                                ps[:, :n],
                                diag[ic][:, t, :],
                                xflat[:, base : base + n],
                                start=(t == 0),
                                stop=(t == 8),
                            )
                        nc.scalar.copy(out=zflat[:, off : off + n], in_=ps[:, :n])
                    z = zt[:, :zrows, :]
                    if z0 <= topq < z0 + zrows:
                        blend(z[:, topq - z0 : topq - z0 + 1, : W - 2],
                              z[:, tops - z0 : tops - z0 + 1, : W - 2], imask_t, W - 2)
                    if z0 <= botq < z0 + zrows:
                        blend(z[:, botq - z0 : botq - z0 + 1, : W - 2],
                              z[:, bots - z0 : bots - z0 + 1, : W - 2], mask_t, W - 2)
                    nc.vector.tensor_copy(
                        out=z[:, :, ctop : ctop + 1], in_=z[:, :, cts : cts + 1]
                    )
                    nc.vector.tensor_copy(
                        out=z[:, :, cbot : cbot + 1], in_=z[:, :, cbs : cbs + 1]
                    )
                    out.append(z)
                return out

            def conv_block(psum, srcs, oc):
                n = 0
                total = 18 * len(srcs)
                for tiles, wt, r0, c0, rows, cols in srcs:
                    for t, (dy, dx) in enumerate(TAPS):
                        for ic in range(2):
                            nc.tensor.matmul(
                                psum,
                                wt[ic][:, t, oc, :],
                                tiles[ic][:, r0 + dy : r0 + dy + rows,
                                          c0 + dx : c0 + dx + cols],
                                start=(n == 0),
                                stop=(n == total - 1),
                            )
                            n += 1

            def band_fixups(sb, topq, tops, botq, bots, ctop, cts, cbot, cbs, cols):
                for ic in range(2):
                    blend(sb[ic][:, topq : topq + 1, :],
                          sb[ic][:, tops : tops + 1, :], imask_t, cols)
                    blend(sb[ic][:, botq : botq + 1, :],
                          sb[ic][:, bots : bots + 1, :], mask_t, cols)
                    nc.vector.tensor_copy(
                        out=sb[ic][:, :, ctop : ctop + 1],
                        in_=sb[ic][:, :, cts : cts + 1],
                    )
                    nc.vector.tensor_copy(
                        out=sb[ic][:, :, cbot : cbot + 1],
                        in_=sb[ic][:, :, cbs : cbs + 1],
                    )

            # ================= LF branch =================
            if STAGES < 2:
                dbg = persist.tile([128, 2048], f32, tag="dbg", name="dbg")
                nc.vector.memset(dbg, 0.0)
                d3 = dbg.rearrange("c (r w) -> c r w", w=OC_)
                for oc in range(2):
                    for r0 in range(0, OR_, 16):
                        nc.sync.dma_start(
                            out=out_hf[oc * 128 : (oc + 1) * 128, r0 : r0 + 16, :],
                            in_=d3)
                d4 = dbg[:, : 16 * OLC].rearrange("c (r w) -> c r w", w=OLC)
                for oc in range(2):
                    for r0 in range(0, OLR, 16):
                        nc.sync.dma_start(
                            out=out_lf[oc * 128 : (oc + 1) * 128, r0 : r0 + 16, :],
                            in_=d4)
                raise _StopBuild()
            ws2_l, diag_l, wp_l, bv_l, S_l = kp_A(
                "sl", "wlsw", "wlpw", "wlbw", "lsb", "lpb", "lbb", "l"
            )
            bias2_l, wadal = kp_B("wadal", "alb", wp_l, bv_l, S_l, m_l, inv_l, "l")

            # hf stats stream first (inv_h gates the ada_h stationary),
            # then the hf band
            load_band(xh, xh_sb, XR, XC, bpool, 7)
            sums_h = stats_stream(xh_ot, 128 * 128, bpool)
            m_h, inv_h = stats(sums_h, xh_sb, (4, 68, 4, 132), 128 * 128, bpool)
            ws2_h, diag_h, wp_h, bv_h, S_h = kp_A(
                "sh", "whsw", "whpw", "whbw", "hsb", "hpb", "hbb", "h",
                want_diag=True,
            )

            if STAGES < 3:
                raise _StopBuild()
            zl_fix = (1, 3, 34, 32, 1, 3, 66, 64)
            for t0 in range(3):
                z0 = 12 * t0
                zt = dw_tile(xl_sb, ws2_l, z0, min(14, ZLR - z0), XLC, "l", zl_fix)
                if STAGES < 4:
                    continue
                for b in range(2 * t0, min(2 * t0 + 2, 6)):
                    r0 = 6 * b
                    rb = min(6, LR - r0)
                    q0 = r0 - 12 * t0
                    for oc in range(2):
                        ps = mmp.tile([128, 6, LC], f32, tag="mm", name="mm")
                        p = ps[:, :rb, :]
                        conv_block(p, [(zt, wadal, q0, 0, rb, LC)], oc)
                        nc.scalar.activation(
                            out=lf_sb[oc][:, r0 : r0 + rb, :], in_=p,
                            func=AF.Lrelu, bias=bias2_l[oc], scale=1.0, alpha=SLOPE,
                        )
            if STAGES < 4:
                raise _StopBuild()
            band_fixups(lf_sb, 0, 2, 33, 31, 0, 2, 65, 63, LC)

            # ================= HF branch =================
            bias2_h, wadah = kp_B("wadah", "ahb", wp_h, bv_h, S_h, m_h, inv_h, "h")

            zh_fix = (2, 4, 67, 65, 2, 4, 131, 129)
            for t0 in range(6):
                z0 = 12 * t0
                zrows = min(14, ZR - z0)
                if t0 in (0, 2, 4):
                    zt = dw_tile_pe(xh_sb, diag_h, z0, zrows, XC, "h", zh_fix)
                else:
                    zt = dw_tile(xh_sb, ws2_h, z0, zrows, XC, "h", zh_fix)
                for b in range(4 * t0, min(4 * t0 + 4, 23)):
                    r0 = 3 * b
                    rb = min(3, HR - r0)
                    q0 = r0 - 12 * t0
                    for oc in range(2):
                        ps = mmp.tile([128, 3, HC], f32, tag="mm", name="mm")
                        p = ps[:, :rb, :]
                        conv_block(p, [(zt, wadah, q0, 0, rb, HC)], oc)
                        nc.scalar.activation(
                            out=hf_sb[oc][:, r0 : r0 + rb, :], in_=p,
                            func=AF.Lrelu, bias=bias2_h[oc], scale=1.0, alpha=SLOPE,
                        )
            band_fixups(hf_sb, 1, 3, 66, 64, 1, 3, 130, 128, HC)

            # ================= cross-frequency fusion =================
            if STAGES < 5:
                raise _StopBuild()
            wh2h = wset("wh2h")
            wl2h = wset("wl2h")

            # special up rows: u=0 -> lf[1]+mask*(lf[0]-lf[1]);
            #                  u=65 -> lf[33]+mask*(lf[32]-lf[33])
            sprow = {}
            for key, ja, jb in (("r0", 1, 0), ("r65", 33, 32)):
                rows = []
                for ic in range(2):
                    d = small.tile([128, 1, LC], f32, tag=f"upd{key}{ic}")
                    nc.vector.tensor_sub(
                        out=d, in0=lf_sb[ic][:, jb : jb + 1, :],
                        in1=lf_sb[ic][:, ja : ja + 1, :],
                    )
                    r = small.tile([128, 1, LC], bf16, tag=f"upr{key}{ic}")
                    nc.vector.scalar_tensor_tensor(
                        out=r, in0=d, scalar=mask_t,
                        in1=lf_sb[ic][:, ja : ja + 1, :],
                        op0=OP.mult, op1=OP.add,
                    )
                    rows.append(r)
                sprow[key] = rows

            def up_cols(dst_rows, src_rows):
                # dst [128, n, 130] <- src [128, n, 64] column-doubling w/ edges
                nc.vector.tensor_copy(out=dst_rows[:, :, 1:129:2], in_=src_rows)
                nc.vector.tensor_copy(out=dst_rows[:, :, 2:130:2], in_=src_rows)
                nc.vector.tensor_copy(
                    out=dst_rows[:, :, 0:1], in_=src_rows[:, :, 0:1]
                )
                nc.vector.tensor_copy(
                    out=dst_rows[:, :, 129:130], in_=src_rows[:, :, 63:64]
                )

            def build_up_tile(g):
                tiles = []
                u0 = 8 * g
                for ic in range(2):
                    ut = upp.tile([128, UPT, UC], bf16, tag="up")
                    ev = [i for i in range(0, UPT, 2) if not (g == 0 and i == 0)]
                    od = [i for i in range(1, UPT, 2) if not (g == 7 and i == 9)]
                    for phase in (ev, od):
                        i0, cnt = phase[0], len(phase)
                        j0 = (u0 + i0 - 1) // 2 + 1
                        dst = ut[:, i0 : i0 + 2 * cnt - 1 : 2, :]
                        src = lf_sb[ic][:, j0 : j0 + cnt, 1:65]
                        up_cols(dst, src)
                    if g == 0:
                        up_cols(ut[:, 0:1, :], sprow["r0"][ic][:, :, 1:65])
                    if g == 7:
                        up_cols(ut[:, 9:10, :], sprow["r65"][ic][:, :, 1:65])
                    tiles.append(ut)
                return tiles

            if STAGES < 6:
                for g in range(8):
                    build_up_tile(g)
                raise _StopBuild()
            up_tiles = {}
            for r in range(16):
                g = r // 2
                if g not in up_tiles:
                    up_tiles[g] = build_up_tile(g)
                u_local = 4 * r - 8 * g
                for oc in range(2):
                    ps = mmp.tile([128, 4, OC_], f32, tag="mm")
                    conv_block(
                        ps,
                        [
                            (hf_sb, wh2h, 4 * r + 1, 1, 4, OC_),
                            (up_tiles[g], wl2h, u_local, 0, 4, OC_),
                        ],
                        oc,
                    )
                    stg = outp.tile([128, 4, OC_], f32, tag="ostg")
                    nc.scalar.activation(
                        out=stg, in_=ps, func=AF.Lrelu, bias=0.0, scale=1.0,
                        alpha=SLOPE,
                    )
                    nc.sync.dma_start(
                        out=out_hf[oc * 128 : (oc + 1) * 128, 4 * r : 4 * r + 4, :],
                        in_=stg,
                    )

            # avgpool of hf (0.25 folded into h2l weights host-side)
            for ic in range(2):
                h4 = hf_sb[ic].rearrange("c (r p) (w q) -> c r p w q", p=2, q=2)
                s1 = accp.tile([128, LR, LC], bf16, tag="avt")
                nc.vector.tensor_add(
                    out=s1, in0=h4[:, :, 0, :, 0], in1=h4[:, :, 0, :, 1]
                )
                s2 = accp.tile([128, LR, LC], bf16, tag="avt")
                nc.vector.tensor_add(
                    out=s2, in0=h4[:, :, 1, :, 0], in1=h4[:, :, 1, :, 1]
                )
                nc.vector.tensor_add(out=avg_sb[ic], in0=s1, in1=s2)
            band_fixups(avg_sb, 0, 2, 33, 31, 0, 2, 65, 63, LC)

            wl2l = wset("wl2l")
            wh2l = wset("wh2l")
            for b in range(6):
                r0 = 6 * b
                rb = min(6, OLR - r0)
                for oc in range(2):
                    ps = mmp.tile([128, 6, OLC], f32, tag="mm")
                    p = ps[:, :rb, :]
                    conv_block(
                        p,
                        [
                            (lf_sb, wl2l, r0, 0, rb, OLC),
                            (avg_sb, wh2l, r0, 0, rb, OLC),
                        ],
                        oc,
                    )
                    stg = outp.tile([128, 6, OLC], f32, tag="ostg2")
                    sg = stg.rearrange("c r w -> c (r w)")[:, : rb * OLC].rearrange(
                        "c (r w) -> c r w", w=OLC)
                    nc.scalar.activation(
                        out=sg, in_=p, func=AF.Lrelu, bias=0.0, scale=1.0,
                        alpha=SLOPE,
                    )
                    nc.sync.dma_start(
                        out=out_lf[oc * 128 : (oc + 1) * 128, r0 : r0 + rb, :],
                        in_=sg,
                    )

    if SPLIT:
        _split_multi_waits(nc, mybir)
    return nc


def _shard(inputs):
    f = lambda k: np.ascontiguousarray(np.asarray(inputs[k], dtype=np.float32))
    c_hf, c_lf, s_hf, s_lf = f("c_hf"), f("c_lf"), f("s_hf"), f("s_lf")
    xhp = np.pad(c_hf, ((0, 0), (0, 0), (4, 4), (4, 4)), mode="reflect")
    xlp = np.pad(c_lf, ((0, 0), (0, 0), (3, 3), (3, 3)), mode="reflect")
    shp = np.pad(s_hf, ((0, 0), (0, 0), (1, 1), (1, 1)), mode="reflect")
    slp = np.pad(s_lf, ((0, 0), (0, 0), (1, 1), (1, 1)), mode="reflect")

    w9 = lambda k, s=1.0: np.ascontiguousarray(
        f(k).reshape(C, C, 9).transpose(1, 2, 0) * s
    )  # [cin, tap, cout]
    wT = lambda k, s=1.0: np.ascontiguousarray(f(k).reshape(C, C).T * s)
    col = lambda k: np.ascontiguousarray(f(k).reshape(C, 1))

    shared = {
        "whsw": w9("h_sw"), "wlsw": w9("l_sw"),
        "wadah": w9("ada_h_w"), "wadal": w9("ada_l_w"),
        "wh2h": w9("h2h"), "wl2h": w9("l2h"),
        "wl2l": w9("l2l"), "wh2l": w9("h2l", 0.25),
        "w1all": np.ascontiguousarray(np.concatenate(
            [wT("h_pw", 1 / 9.0), wT("h_bw", 1 / 9.0),
             wT("l_pw", 1 / 9.0), wT("l_bw", 1 / 9.0)], axis=1)),
        "ball": np.ascontiguousarray(np.stack(
            [f(k).reshape(C) for k in ("h_sb", "h_pb", "h_bb", "ada_h_b",
                                       "l_sb", "l_pb", "l_bb", "ada_l_b")],
            axis=1)),
    }
    maps = []
    for core in range(NCORES):
        s, h = core // 2, core % 2
        m = dict(shared)
        oh = 1 - h
        m["xh"] = np.ascontiguousarray(xhp[s][:, 64 * h : 64 * h + XR, :XC])
        m["xh_ot"] = np.ascontiguousarray(c_hf[s][:, 64 * oh : 64 * oh + 64, :])
        m["xl"] = np.ascontiguousarray(xlp[s][:, 32 * h : 32 * h + XLR, :XLC])
        m["xl_ot"] = np.ascontiguousarray(c_lf[s][:, 32 * oh : 32 * oh + 32, :])
        m["sall"] = np.ascontiguousarray(np.stack([shp[s], slp[s]], axis=1))
        m["maskp"] = np.full((128, 1), float(h), np.float32)
        m["identp"] = np.eye(128, dtype=np.float32)
        maps.append(m)
    return maps


def _run(in_maps, trace=False, **kw):
    from concourse.bass_utils import run_bass_kernel_spmd

    if "nc" not in _CACHE:
        _CACHE["nc"] = _build_nc()
    return run_bass_kernel_spmd(
        _CACHE["nc"], in_maps, core_ids=list(range(NCORES)), trace=trace, **kw
    )


def kernel(**inputs):
    res = _run(_shard(inputs))
    hf = np.zeros((B, C, 128, 128), np.float32)
    lf = np.zeros((B, C, 64, 64), np.float32)
    for core in range(NCORES):
        s, h = core // 2, core % 2
        hf[s][:, 64 * h : 64 * h + OR_, :] = res.results[core]["out_hf"]
        lf[s][:, 32 * h : 32 * h + OLR, :] = res.results[core]["out_lf"]
    return hf, lf


# revision 35
# speedup vs baseline: 1.1494x; 1.1494x over previous
# AdaOctConv distributed Trainium2 kernel (8 NeuronCores, SPMD, no collectives).
#
# Sharding: 4 samples x 2 spatial halves = 8 cores. Each core computes a
# 64-row band of hf_out and a 32-row band of lf_out of one sample. Internal
# split halos come in via host-side overlapped shards; image-border halos are
# reflect-copies of computed interior rows, applied on device as mask-blended
# row/col fixups so all 8 cores share one SPMD graph.
#
# Math folding (per channel c, per sample):
#   instance_norm -> depthwise3x3(per-sample w) -> *w_point + bias  ==
#   depthwise3x3(raw x; ws*(wp*rsqrt(var+eps))) + beta, with beta folded into
#   the following dense conv's bias. The heavy work is six 3x3 dense convs,
#   each computed as 9 shift-matmuls (bf16 in, fp32 PSUM accumulation).
import sys

for _p in ("/opt/trn_rl_repo",):
    if _p not in sys.path:
        sys.path.append(_p)

import os

import numpy as np

B, C = 4, 256
EPS, SLOPE = 1e-5, 0.01
NCORES = 8

XR, XC = 72, 136      # x band (reflect-padded +-4)
ZR, ZC = 70, 134      # depthwise output band
HR, HC = 68, 132      # hf band
OR_, OC_ = 64, 128    # hf output band
XLR, XLC = 38, 70
ZLR, ZLC = 36, 68
LR, LC = 34, 66
OLR, OLC = 32, 64
UR, UC = 66, 130      # upsampled-lf band
UPT = 10              # rows per up tile (8 tiles of [8g, 8g+10))

_CACHE = {}


def _split_multi_waits(nc, mybir):
    """This walrus build accepts at most one sync-wait per instruction;
    hoist extra waits onto single-wait NOPs on the same engine."""
    for fn in nc.m.functions:
        for bb in fn.blocks:
            new = []
            for ins in bb.instructions:
                si = ins.sync_info
                if si and si.on_wait and len(si.on_wait) > 1:
                    waits = list(si.on_wait)
                    for w in waits[:-1]:
                        nop = mybir.InstNoOp(
                            name=f"I-ws-{nc.next_id()}", ins=[], outs=[]
                        )
                        nop.engine = ins.engine
                        nop.sync_info = mybir.SyncInfo(on_wait=[w], on_update=[])
                        new.append(nop)
                    ins.sync_info = mybir.SyncInfo(
                        on_wait=[waits[-1]], on_update=list(si.on_update or [])
                    )
                new.append(ins)
            bb.instructions[:] = new


def _build_nc():
    import contextlib

    import concourse.bass as bass
    import concourse.mybir as mybir
    import concourse.tile as tile

    f32 = mybir.dt.float32
    bf16 = mybir.dt.bfloat16
    AF = mybir.ActivationFunctionType
    OP = mybir.AluOpType
    AX = mybir.AxisListType

    STAGES = int(os.environ.get("ADAOCT_STAGES", "9"))
    SPLIT = os.environ.get("ADAOCT_SPLIT", "1") == "1"

    nc = bass.Bass()

    def par(name, shape, out=False):
        return nc.declare_dram_parameter(name, list(shape), f32, isOutput=out)

    xh = par("xh", (C, XR, XC))
    W1N = ("whpw", "whbw", "wlpw", "wlbw")
    BN = ("hsb", "hpb", "hbb", "ahb", "lsb", "lpb", "lbb", "alb")
    xh_ot = par("xh_ot", (C, 64, 128))
    xl = par("xl", (C, XLR, XLC))
    xl_ot = par("xl_ot", (C, 32, 64))
    identp = par("identp", (128, 128))
    maskp = par("maskp", (128, 1))
    wsets = {
        n: par(n, (C, 9, C))
        for n in ("whsw", "wlsw", "wadal", "wadah", "wh2h", "wl2h", "wl2l", "wh2l")
    }
    w1all = par("w1all", (C, 4 * C))  # whpw|whbw|wlpw|wlbw along cols
    ball = par("ball", (C, 8))  # hsb hpb hbb ahb lsb lpb lbb alb
    sall = par("sall", (C, 2, 5, 5))  # sh | sl
    out_hf = par("out_hf", (C, OR_, OC_), out=True)
    out_lf = par("out_lf", (C, OLR, OLC), out=True)

    TAPS = [(dy, dx) for dy in range(3) for dx in range(3)]

    class _StopBuild(Exception):
        pass

    with tile.TileContext(nc) as tc:
        es = contextlib.ExitStack()
        with es, contextlib.suppress(_StopBuild):
            persist = es.enter_context(tc.tile_pool(name="persist", bufs=1))
            small = es.enter_context(tc.tile_pool(name="small", bufs=1))
            wpool = es.enter_context(tc.tile_pool(name="wpool", bufs=4))
            wstg = es.enter_context(tc.tile_pool(name="wstg", bufs=2))
            psmall = es.enter_context(tc.tile_pool(name="psmall", bufs=2, space="PSUM"))
            mmp = es.enter_context(tc.tile_pool(name="mmp", bufs=6, space="PSUM"))
            accp = es.enter_context(tc.tile_pool(name="accp", bufs=1))
            z2p = es.enter_context(tc.tile_pool(name="z2p", bufs=4))
            upp = es.enter_context(tc.tile_pool(name="upp", bufs=2))
            outp = es.enter_context(tc.tile_pool(name="outp", bufs=2))
            fixp = es.enter_context(tc.tile_pool(name="fixp", bufs=1))

            # ---------- persistent tensors ----------
            xh_sb = [persist.tile([128, XR, XC], bf16, tag=f"xh{i}") for i in range(2)]
            xl_sb = [persist.tile([128, XLR, XLC], bf16, tag=f"xl{i}") for i in range(2)]
            hf_sb = [persist.tile([128, HR, HC], bf16, tag=f"hf{i}") for i in range(2)]
            lf_sb = [persist.tile([128, LR, LC], bf16, tag=f"lf{i}") for i in range(2)]
            avg_sb = [persist.tile([128, LR, LC], bf16, tag=f"av{i}") for i in range(2)]

            eps_t = small.tile([128, 1], f32, tag="eps")
            nc.vector.memset(eps_t, EPS)
            mask_t = small.tile([128, 1], f32, tag="mask")
            nc.sync.dma_start(out=mask_t, in_=maskp[:, :])
            imask_t = small.tile([128, 1], f32, tag="imask")
            nc.scalar.activation(
                out=imask_t, in_=mask_t, func=AF.Copy, bias=1.0, scale=-1.0
            )

            identf = small.tile([128, 128], f32, tag="identf", name="identf")
            nc.sync.dma_start(out=identf, in_=identp[:, :])
            ident_b = small.tile([128, 128], bf16, tag="identb", name="identb")
            nc.scalar.copy(out=ident_b, in_=identf)

            def blend(cur, ref, sel, cols):
                """cur = cur + sel*(ref - cur) over a [128, 1, cols] slice."""
                d = fixp.tile([128, 1, 144], f32, tag="fixd")
                dd = d[:, :, :cols]
                nc.vector.tensor_sub(out=dd, in0=ref, in1=cur)
                nc.vector.scalar_tensor_tensor(
                    out=cur, in0=dd, scalar=sel, in1=cur, op0=OP.mult, op1=OP.add
                )

            bias_sb = {}
            for i in range(2):
                bt = small.tile([128, 8], f32, tag=f"ball{i}", name=f"ball{i}")
                nc.sync.dma_start(out=bt, in_=ball[i * 128 : (i + 1) * 128, :])
                for j, n in enumerate(BN):
                    bias_sb[(n, i)] = bt[:, j : j + 1]

            w1_sb = {}
            for i in range(2):
                stg = wstg.tile([128, 1152], f32, tag="wstg", name="w1stg")
                nc.sync.dma_start(
                    out=stg[:, : 4 * C].rearrange("c (k o) -> c k o", k=4),
                    in_=w1all[i * 128 : (i + 1) * 128, :].rearrange(
                        "c (k o) -> c k o", k=4),
                )
                t = small.tile([128, 4, C], bf16, tag=f"w1_{i}", name=f"w1_{i}")
                nc.scalar.copy(out=t, in_=stg[:, : 4 * C].rearrange(
                    "c (k o) -> c k o", k=4))
                for j, n in enumerate(W1N):
                    w1_sb[(n, i)] = t[:, j, :]

            sbuf_s = {}
            for i in range(2):
                tf = small.tile([128, 2, 5, 5], f32, tag=f"sf{i}", name=f"sf{i}")
                nc.sync.dma_start(out=tf, in_=sall[i * 128 : (i + 1) * 128, :, :, :])
                tb = small.tile([128, 2, 5, 5], bf16, tag=f"sb{i}", name=f"sb{i}")
                nc.scalar.copy(out=tb, in_=tf)
                for j, nm in enumerate(("sh", "sl")):
                    sbuf_s[(nm, i, "f")] = tf[:, j]
                    sbuf_s[(nm, i, "b")] = tb[:, j]

            # ---------- weight set loading ----------
            wcache = {}

            def wset(name):
                if name in wcache:
                    return wcache[name]
                p = wsets[name]
                tiles = []
                for i in range(2):
                    t = wpool.tile([128, 9, 2, 128], bf16, tag="wset")
                    flat = t.rearrange("c t o q -> c (t o q)")
                    src = p[i * 128 : (i + 1) * 128, :, :].rearrange("c t o -> c (t o)")
                    for j in range(2):
                        stg = wstg.tile([128, 1152], f32, tag="wstg")
                        nc.sync.dma_start(
                            out=stg, in_=src[:, j * 1152 : (j + 1) * 1152]
                        )
                        nc.scalar.copy(out=flat[:, j * 1152 : (j + 1) * 1152], in_=stg)
                    tiles.append(t)
                wcache[name] = tiles
                return tiles

            # ---------- instance-norm stats ----------
            # mean/var over [own band interior (bf16, in SBUF)] + [other
            # half's rows, streamed f32]. Sums via ScalarE accum_out.
            def stats_stream(other, hw, bpool):
                npc = (hw // 2) // 1024
                bn_t = []
                for i in range(2):
                    nm = f"{other.name}{i}"
                    bnst = small.tile([128, 2 * npc, 6], f32, tag=f"bn_{nm}",
                                      name=f"bn_{nm}")
                    flat = other[i * 128 : (i + 1) * 128, :, :].rearrange(
                        "c h w -> c (h w)"
                    )
                    for k in range(npc):
                        pc = bpool.tile([128, 1024], f32, tag="xstg2", name="xstg2")
                        nc.gpsimd.dma_start(
                            out=pc, in_=flat[:, k * 1024 : (k + 1) * 1024]
                        )
                        for j in range(2):
                            nc.vector.bn_stats(
                                out=bnst[:, 2 * k + j, :],
                                in_=pc[:, j * 512 : (j + 1) * 512],
                            )
                    bn_t.append(bnst)
                return bn_t

            def stats(bn_t, band_sb, bint, hw, bpool):
                r0, r1, c0, c1 = bint
                half = hw // 2
                m_t, inv_t = [], []
                for i in range(2):
                    nm = bn_t[i].tensor.name
                    sums = small.tile([128, 5, 2], f32, tag=f"s5_{nm}",
                                      name=f"s5_{nm}")
                    mvs = small.tile([128, 2], f32, tag=f"mvs_{nm}",
                                     name=f"mvs_{nm}")
                    nc.vector.bn_aggr(out=mvs, in_=bn_t[i])
                    # stream half back to (sum, sumsq) space
                    nc.vector.tensor_scalar_mul(out=sums[:, 4, 0:1],
                                                in0=mvs[:, 0:1], scalar1=float(half))
                    msq = small.tile([128, 1], f32, tag=f"msq_{nm}",
                                     name=f"msq_{nm}")
                    nc.vector.tensor_mul(out=msq, in0=mvs[:, 0:1], in1=mvs[:, 0:1])
                    nc.vector.tensor_add(out=msq, in0=msq, in1=mvs[:, 1:2])
                    nc.vector.tensor_scalar_mul(out=sums[:, 4, 1:2], in0=msq,
                                                scalar1=float(half))
                    rows = r1 - r0
                    q = rows // 4
                    for j in range(4):
                        seg = band_sb[i][:, r0 + j * q : r0 + (j + 1) * q, c0:c1]
                        scr = bpool.tile([128, 2048], bf16, tag="scr", name="scr",
                                         bufs=1)
                        sg = scr[:, : seg.free_size()]
                        nc.scalar.activation(out=sg, in_=seg, func=AF.Copy,
                                             accum_out=sums[:, j, 0:1])
                        nc.scalar.activation(out=sg, in_=seg, func=AF.Square,
                                             accum_out=sums[:, j, 1:2])
                    mv = small.tile([128, 2], f32, tag=f"mv_{nm}", name=f"mv_{nm}")
                    nc.vector.tensor_reduce(
                        out=mv, in_=sums.rearrange("c n k -> c k n"),
                        axis=AX.X, op=OP.add)
                    m = small.tile([128, 1], f32, tag=f"m_{nm}", name=f"m_{nm}")
                    nc.vector.tensor_scalar_mul(out=m, in0=mv[:, 0:1],
                                                scalar1=1.0 / hw)
                    var = small.tile([128, 1], f32, tag=f"v_{nm}", name=f"v_{nm}")
                    nc.vector.tensor_scalar_mul(out=var, in0=mv[:, 1:2],
                                                scalar1=1.0 / hw)
                    msqt = small.tile([128, 1], f32, tag=f"mq_{nm}", name=f"mq_{nm}")
                    nc.vector.tensor_mul(out=msqt, in0=m, in1=m)
                    nc.vector.tensor_sub(out=var, in0=var, in1=msqt)
                    inv = small.tile([128, 1], f32, tag=f"i_{nm}", name=f"i_{nm}")
                    nc.scalar.activation(
                        out=inv, in_=var, func=AF.Sqrt, bias=eps_t, scale=1.0
                    )
                    nc.vector.reciprocal(out=inv, in_=inv)
                    m_t.append(m)
                    inv_t.append(inv)
                return m_t, inv_t

            def load_band(param, dst, rows, cols, bpool, rchunk):
                for i in range(2):
                    src = param[i * 128 : (i + 1) * 128, :, :]
                    for r0 in range(0, rows, rchunk):
                        r1 = min(r0 + rchunk, rows)
                        stg = bpool.tile([128, 3264], f32, tag="xstg")
                        s3 = stg[:, : (r1 - r0) * cols].rearrange(
                            "c (r w) -> c r w", w=cols
                        )
                        nc.gpsimd.dma_start(out=s3, in_=src[:, r0:r1, :])
                        nc.scalar.copy(out=dst[i][:, r0:r1, :], in_=s3)

            bpool = es.enter_context(tc.tile_pool(name="bpool", bufs=2))
            wset("wlsw")
            wset("wadal")
            sums_l = stats_stream(xl_ot, 64 * 64, bpool)
            load_band(xl, xl_sb, XLR, XLC, bpool, 14)
            m_l, inv_l = stats(sums_l, xl_sb, (3, 35, 3, 67), 64 * 64, bpool)

            # ---------- kernel_predict + folds ----------
            # ws2 = ws*wp (stats-free; inv is folded into the dense-conv
            # stationary later). beta = b - (wp*inv)*m*S.
            def kp_A(skey, wsw_name, w1pw, w1bw, bsb, bpb, bbb, tag, want_diag=False):
                wsw = wset(wsw_name)
                s_f = [sbuf_s[(skey, i, "f")] for i in range(2)]
                s_b = [sbuf_s[(skey, i, "b")] for i in range(2)]
                pooled_b = []
                for i in range(2):
                    pf = small.tile([128, 1], f32, tag=f"poo{tag}{i}", name=f"poo{tag}{i}")
                    nc.vector.reduce_sum(out=pf, in_=s_f[i][:, 1:4, 1:4], axis=AX.XY)
                    pb = small.tile([128, 1], bf16, tag=f"poob{tag}{i}", name=f"poob{tag}{i}")
                    nc.scalar.copy(out=pb, in_=pf)
                    pooled_b.append(pb)

                ws2, wp_t, bv_t, S_t, diag = [], [], [], [], []
                for oc in range(2):
                    ps = psmall.tile([128, 9], f32, tag="ps9", name="ps9")
                    ps3 = ps.rearrange("c (a b) -> c a b", a=3)
                    n = 0
                    for t, (dy, dx) in enumerate(TAPS):
                        for ic in range(2):
                            nc.tensor.matmul(
                                ps3,
                                wsw[ic][:, t, oc, :],
                                s_b[ic][:, dy : dy + 3, dx : dx + 3],
                                start=(n == 0),
                                stop=(n == 17),
                            )
                            n += 1
                    wf = small.tile([128, 9], f32, tag=f"ws{tag}{oc}", name=f"ws{tag}{oc}")
                    nc.scalar.activation(out=wf, in_=ps, func=AF.Identity,
                                         bias=bias_sb[(bsb, oc)], scale=1.0)

                    ps1 = psmall.tile([128, 9], f32, tag="ps9", name="ps9")
                    for ic in range(2):
                        nc.tensor.matmul(
                            ps1[:, 0:1],
                            w1_sb[(w1pw, ic)][:, oc * 128 : (oc + 1) * 128],
                            pooled_b[ic],
                            start=(ic == 0),
                            stop=(ic == 1),
                        )
                    wp = small.tile([128, 1], f32, tag=f"wp{tag}{oc}", name=f"wp{tag}{oc}")
                    nc.scalar.activation(out=wp, in_=ps1[:, 0:1], func=AF.Identity,
                                         bias=bias_sb[(bpb, oc)], scale=1.0)
                    wp_t.append(wp)

                    ps2 = psmall.tile([128, 9], f32, tag="ps9", name="ps9")
                    for ic in range(2):
                        nc.tensor.matmul(
                            ps2[:, 0:1],
                            w1_sb[(w1bw, ic)][:, oc * 128 : (oc + 1) * 128],
                            pooled_b[ic],
                            start=(ic == 0),
                            stop=(ic == 1),
                        )
                    bv = small.tile([128, 1], f32, tag=f"bv{tag}{oc}", name=f"bv{tag}{oc}")
                    nc.scalar.activation(out=bv, in_=ps2[:, 0:1], func=AF.Identity,
                                         bias=bias_sb[(bbb, oc)], scale=1.0)
                    bv_t.append(bv)

                    w2 = small.tile([128, 9], f32, tag=f"w2{tag}{oc}", name=f"w2{tag}{oc}")
                    nc.scalar.activation(out=w2, in_=wf, func=AF.Identity,
                                         bias=0.0, scale=wp)
                    ws2.append(w2)
                    s_s = small.tile([128, 1], f32, tag=f"S{tag}{oc}", name=f"S{tag}{oc}")
                    nc.vector.reduce_sum(out=s_s, in_=wf, axis=AX.X)
                    S_t.append(s_s)
                    if want_diag:
                        dg = small.tile([128, 9, 128], bf16, tag=f"dg{tag}{oc}", name=f"dg{tag}{oc}")
                        for t in range(9):
                            nc.scalar.activation(
                                out=dg[:, t, :], in_=ident_b, func=AF.Identity,
                                bias=0.0, scale=w2[:, t : t + 1])
                        diag.append(dg)
                return ws2, diag, wp_t, bv_t, S_t

            def kp_B(wada_name, bada, wp_t, bv_t, S_t, m_t, inv_t, tag):
                """bias2 from unscaled wada, then scale wada in place by
                inv[cin] (per-partition)."""
                wada = wset(wada_name)
                beta_b = []
                for oc in range(2):
                    tmp = small.tile([128, 1], f32, tag=f"t1{tag}{oc}", name=f"t1{tag}{oc}")
                    nc.scalar.activation(out=tmp, in_=wp_t[oc], func=AF.Identity,
                                         bias=0.0, scale=inv_t[oc])
                    nc.scalar.activation(out=tmp, in_=tmp, func=AF.Identity,
                                         bias=0.0, scale=m_t[oc])
                    nc.scalar.activation(out=tmp, in_=tmp, func=AF.Identity,
                                         bias=0.0, scale=S_t[oc])
                    nc.scalar.mul(out=tmp, in_=tmp, mul=-1.0)
                    bb_ = small.tile([128, 1], bf16, tag=f"beb{tag}{oc}", name=f"beb{tag}{oc}")
                    nc.scalar.activation(out=bb_, in_=bv_t[oc], func=AF.Identity,
                                         bias=tmp, scale=1.0)
                    beta_b.append(bb_)
                bias2 = []
                for oc in range(2):
                    ps = psmall.tile([128, 9], f32, tag="ps9", name="ps9")
                    n = 0
                    for t in range(9):
                        for ic in range(2):
                            nc.tensor.matmul(
                                ps[:, 0:1],
                                wada[ic][:, t, oc, :],
                                beta_b[ic],
                                start=(n == 0),
                                stop=(n == 17),
                            )
                            n += 1
                    b2 = small.tile([128, 1], f32, tag=f"b2{tag}{oc}", name=f"b2{tag}{oc}")
                    nc.scalar.activation(out=b2, in_=ps[:, 0:1], func=AF.Identity,
                                         bias=bias_sb[(bada, oc)], scale=1.0)
                    bias2.append(b2)
                for ic in range(2):
                    wf_ = wada[ic].rearrange("c t o q -> c (t o q)")
                    nc.scalar.activation(out=wf_, in_=wf_, func=AF.Identity,
                                         bias=0.0, scale=inv_t[ic])
                return bias2, wada

            # ---------- depthwise (9-tap MAC on the Vector engine) ----------
            def dw_tile(x_sb, ws2, z0, zrows, W, tag, fix):
                """z[c, q, w] = sum_t ws2[c,t] * x[c, q+dy, w+dx]; cols
                [0, W-2) valid. Returns [128, zrows, W] bf16 views."""
                topq, tops, botq, bots, ctop, cts, cbot, cbs = fix
                zc = W - 2
                out = []
                for ic in range(2):
                    acc = accp.tile([128, 14, 134], bf16, tag="acc", name="acc")
                    a = acc[:, :zrows, :zc]
                    zt = z2p.tile([128, 14, zc], bf16, tag=f"z2{tag}", name=f"z2{tag}")
                    z = zt[:, :zrows, :]
                    zv = z
                    for t, (dy, dx) in enumerate(TAPS):
                        in0 = x_sb[ic][:, z0 + dy : z0 + dy + zrows, dx : dx + zc]
                        sc = ws2[ic][:, t : t + 1]
                        if t == 0:
                            nc.vector.tensor_scalar_mul(out=a, in0=in0, scalar1=sc)
                        elif t < 8:
                            nc.vector.scalar_tensor_tensor(
                                out=a, in0=in0, scalar=sc, in1=a,
                                op0=OP.mult, op1=OP.add,
                            )
                        else:
                            nc.vector.scalar_tensor_tensor(
                                out=zv, in0=in0, scalar=sc, in1=a,
                                op0=OP.mult, op1=OP.add,
                            )
                    if z0 <= topq < z0 + zrows:
                        blend(z[:, topq - z0 : topq - z0 + 1, :zc],
                              z[:, tops - z0 : tops - z0 + 1, :zc], imask_t, zc)
                    if z0 <= botq < z0 + zrows:
                        blend(z[:, botq - z0 : botq - z0 + 1, :zc],
                              z[:, bots - z0 : bots - z0 + 1, :zc], mask_t, zc)
                    nc.vector.tensor_copy(
                        out=z[:, :, ctop : ctop + 1], in_=z[:, :, cts : cts + 1]
                    )
                    nc.vector.tensor_copy(
                        out=z[:, :, cbot : cbot + 1], in_=z[:, :, cbs : cbs + 1]
                    )
                    out.append(z)
                return out

            def dw_tile_pe(x_sb, diag, z0, zrows, W, tag, fix):
                """Same contract as dw_tile, but computed on the TensorEngine
                as 9 accumulated diag-matmuls over flat 512-col windows."""
                topq, tops, botq, bots, ctop, cts, cbot, cbs = fix
                L = zrows * W - 2
                out = []
                for ic in range(2):
                    zt = z2p.tile([128, 14, W], bf16, tag=f"z2{tag}", name=f"z2p{tag}")
                    zflat = zt.rearrange("c r w -> c (r w)")
                    xflat = x_sb[ic].rearrange("c r w -> c (r w)")
                    for off in range(0, L, 512):
                        n = min(512, L - off)
                        ps = mmp.tile([128, 512], f32, tag="mm", name="zps")
                        for t, (dy, dx) in enumerate(TAPS):
                            base = (z0 + dy) * W + dx + off
                            nc.tensor.matmul(
                                ps[:, :n],
                                diag[ic][:, t, :],
                                xflat[:, base : base + n],
                                start=(t == 0),
                                stop=(t == 8),
                            )
                        nc.scalar.copy(out=zflat[:, off : off + n], in_=ps[:, :n])
                    z = zt[:, :zrows, :]
                    if z0 <= topq < z0 + zrows:
                        blend(z[:, topq - z0 : topq - z0 + 1, : W - 2],
                              z[:, tops - z0 : tops - z0 + 1, : W - 2], imask_t, W - 2)
                    if z0 <= botq < z0 + zrows:
                        blend(z[:, botq - z0 : botq - z0 + 1, : W - 2],
                              z[:, bots - z0 : bots - z0 + 1, : W - 2], mask_t, W - 2)
                    nc.vector.tensor_copy(
                        out=z[:, :, ctop : ctop + 1], in_=z[:, :, cts : cts + 1]
                    )
                    nc.vector.tensor_copy(
                        out=z[:, :, cbot : cbot + 1], in_=z[:, :, cbs : cbs + 1]
                    )
                    out.append(z)
                return out

            def conv_block(psum, srcs, oc):
                n = 0
                total = 18 * len(srcs)
                for tiles, wt, r0, c0, rows, cols in srcs:
                    for t, (dy, dx) in enumerate(TAPS):
                        for ic in range(2):
                            nc.tensor.matmul(
                                psum,
                                wt[ic][:, t, oc, :],
                                tiles[ic][:, r0 + dy : r0 + dy + rows,
                                          c0 + dx : c0 + dx + cols],
                                start=(n == 0),
                                stop=(n == total - 1),
                            )
                            n += 1

            def band_fixups(sb, topq, tops, botq, bots, ctop, cts, cbot, cbs, cols):
                for ic in range(2):
                    blend(sb[ic][:, topq : topq + 1, :],
                          sb[ic][:, tops : tops + 1, :], imask_t, cols)
                    blend(sb[ic][:, botq : botq + 1, :],
                          sb[ic][:, bots : bots + 1, :], mask_t, cols)
                    nc.vector.tensor_copy(
                        out=sb[ic][:, :, ctop : ctop + 1],
                        in_=sb[ic][:, :, cts : cts + 1],
                    )
                    nc.vector.tensor_copy(
                        out=sb[ic][:, :, cbot : cbot + 1],
                        in_=sb[ic][:, :, cbs : cbs + 1],
                    )

            # ================= LF branch =================
            if STAGES < 2:
                dbg = persist.tile([128, 2048], f32, tag="dbg", name="dbg")
                nc.vector.memset(dbg, 0.0)
                d3 = dbg.rearrange("c (r w) -> c r w", w=OC_)
                for oc in range(2):
                    for r0 in range(0, OR_, 16):
                        nc.sync.dma_start(
                            out=out_hf[oc * 128 : (oc + 1) * 128, r0 : r0 + 16, :],
                            in_=d3)
                d4 = dbg[:, : 16 * OLC].rearrange("c (r w) -> c r w", w=OLC)
                for oc in range(2):
                    for r0 in range(0, OLR, 16):
                        nc.sync.dma_start(
                            out=out_lf[oc * 128 : (oc + 1) * 128, r0 : r0 + 16, :],
                            in_=d4)
                raise _StopBuild()
            ws2_l, diag_l, wp_l, bv_l, S_l = kp_A(
                "sl", "wlsw", "wlpw", "wlbw", "lsb", "lpb", "lbb", "l",
                want_diag=True,
            )
            bias2_l, wadal = kp_B("wadal", "alb", wp_l, bv_l, S_l, m_l, inv_l, "l")

            # hf stats stream first (inv_h gates the ada_h stationary),
            # then the hf band
            sums_h = stats_stream(xh_ot, 128 * 128, bpool)
            load_band(xh, xh_sb, XR, XC, bpool, 7)
            m_h, inv_h = stats(sums_h, xh_sb, (4, 68, 4, 132), 128 * 128, bpool)
            ws2_h, diag_h, wp_h, bv_h, S_h = kp_A(
                "sh", "whsw", "whpw", "whbw", "hsb", "hpb", "hbb", "h",
                want_diag=True,
            )

            if STAGES < 3:
                raise _StopBuild()
            zl_fix = (1, 3, 34, 32, 1, 3, 66, 64)
            for t0 in range(3):
                z0 = 12 * t0
                zt = dw_tile_pe(xl_sb, diag_l, z0, min(14, ZLR - z0), XLC, "l", zl_fix)
                if STAGES < 4:
                    continue
                for b in range(2 * t0, min(2 * t0 + 2, 6)):
                    r0 = 6 * b
                    rb = min(6, LR - r0)
                    q0 = r0 - 12 * t0
                    for oc in range(2):
                        ps = mmp.tile([128, 6, LC], f32, tag="mm", name="mm")
                        p = ps[:, :rb, :]
                        conv_block(p, [(zt, wadal, q0, 0, rb, LC)], oc)
                        nc.scalar.activation(
                            out=lf_sb[oc][:, r0 : r0 + rb, :], in_=p,
                            func=AF.Lrelu, bias=bias2_l[oc], scale=1.0, alpha=SLOPE,
                        )
            if STAGES < 4:
                raise _StopBuild()
            band_fixups(lf_sb, 0, 2, 33, 31, 0, 2, 65, 63, LC)

            # ================= HF branch =================
            bias2_h, wadah = kp_B("wadah", "ahb", wp_h, bv_h, S_h, m_h, inv_h, "h")

            zh_fix = (2, 4, 67, 65, 2, 4, 131, 129)
            for t0 in range(6):
                z0 = 12 * t0
                zrows = min(14, ZR - z0)
                if t0 in (0, 2, 4):
                    zt = dw_tile_pe(xh_sb, diag_h, z0, zrows, XC, "h", zh_fix)
                else:
                    zt = dw_tile(xh_sb, ws2_h, z0, zrows, XC, "h", zh_fix)
                for b in range(4 * t0, min(4 * t0 + 4, 23)):
                    r0 = 3 * b
                    rb = min(3, HR - r0)
                    q0 = r0 - 12 * t0
                    for oc in range(2):
                        ps = mmp.tile([128, 3, HC], f32, tag="mm", name="mm")
                        p = ps[:, :rb, :]
                        conv_block(p, [(zt, wadah, q0, 0, rb, HC)], oc)
                        nc.scalar.activation(
                            out=hf_sb[oc][:, r0 : r0 + rb, :], in_=p,
                            func=AF.Lrelu, bias=bias2_h[oc], scale=1.0, alpha=SLOPE,
                        )
            band_fixups(hf_sb, 1, 3, 66, 64, 1, 3, 130, 128, HC)

            # ================= cross-frequency fusion =================
            if STAGES < 5:
                raise _StopBuild()
            wh2h = wset("wh2h")
            wl2h = wset("wl2h")

            # special up rows: u=0 -> lf[1]+mask*(lf[0]-lf[1]);
            #                  u=65 -> lf[33]+mask*(lf[32]-lf[33])
            sprow = {}
            for key, ja, jb in (("r0", 1, 0), ("r65", 33, 32)):
                rows = []
                for ic in range(2):
                    d = small.tile([128, 1, LC], f32, tag=f"upd{key}{ic}")
                    nc.vector.tensor_sub(
                        out=d, in0=lf_sb[ic][:, jb : jb + 1, :],
                        in1=lf_sb[ic][:, ja : ja + 1, :],
                    )
                    r = small.tile([128, 1, LC], bf16, tag=f"upr{key}{ic}")
                    nc.vector.scalar_tensor_tensor(
                        out=r, in0=d, scalar=mask_t,
                        in1=lf_sb[ic][:, ja : ja + 1, :],
                        op0=OP.mult, op1=OP.add,
                    )
                    rows.append(r)
                sprow[key] = rows

            def up_cols(dst_rows, src_rows):
                # dst [128, n, 130] <- src [128, n, 64] column-doubling w/ edges
                nc.vector.tensor_copy(out=dst_rows[:, :, 1:129:2], in_=src_rows)
                nc.vector.tensor_copy(out=dst_rows[:, :, 2:130:2], in_=src_rows)
                nc.vector.tensor_copy(
                    out=dst_rows[:, :, 0:1], in_=src_rows[:, :, 0:1]
                )
                nc.vector.tensor_copy(
                    out=dst_rows[:, :, 129:130], in_=src_rows[:, :, 63:64]
                )

            def build_up_tile(g):
                tiles = []
                u0 = 8 * g
                for ic in range(2):
                    ut = upp.tile([128, UPT, UC], bf16, tag="up")
                    ev = [i for i in range(0, UPT, 2) if not (g == 0 and i == 0)]
                    od = [i for i in range(1, UPT, 2) if not (g == 7 and i == 9)]
                    for phase in (ev, od):
                        i0, cnt = phase[0], len(phase)
                        j0 = (u0 + i0 - 1) // 2 + 1
                        dst = ut[:, i0 : i0 + 2 * cnt - 1 : 2, :]
                        src = lf_sb[ic][:, j0 : j0 + cnt, 1:65]
                        up_cols(dst, src)
                    if g == 0:
                        up_cols(ut[:, 0:1, :], sprow["r0"][ic][:, :, 1:65])
                    if g == 7:
                        up_cols(ut[:, 9:10, :], sprow["r65"][ic][:, :, 1:65])
                    tiles.append(ut)
                return tiles

            if STAGES < 6:
                for g in range(8):
                    build_up_tile(g)
                raise _StopBuild()
            up_tiles = {}
            for r in range(16):
                g = r // 2
                if g not in up_tiles:
                    up_tiles[g] = build_up_tile(g)
                u_local = 4 * r - 8 * g
                for oc in range(2):
                    ps = mmp.tile([128, 4, OC_], f32, tag="mm")
                    conv_block(
                        ps,
                        [
                            (hf_sb, wh2h, 4 * r + 1, 1, 4, OC_),
                            (up_tiles[g], wl2h, u_local, 0, 4, OC_),
                        ],
                        oc,
                    )
                    stg = outp.tile([128, 4, OC_], f32, tag="ostg")
                    nc.scalar.activation(
                        out=stg, in_=ps, func=AF.Lrelu, bias=0.0, scale=1.0,
                        alpha=SLOPE,
                    )
                    nc.sync.dma_start(
                        out=out_hf[oc * 128 : (oc + 1) * 128, 4 * r : 4 * r + 4, :],
                        in_=stg,
                    )

            # avgpool of hf (0.25 folded into h2l weights host-side)
            for ic in range(2):
                h4 = hf_sb[ic].rearrange("c (r p) (w q) -> c r p w q", p=2, q=2)
                s1 = accp.tile([128, LR, LC], bf16, tag="avt")
                nc.vector.tensor_add(
                    out=s1, in0=h4[:, :, 0, :, 0], in1=h4[:, :, 0, :, 1]
                )
                s2 = accp.tile([128, LR, LC], bf16, tag="avt")
                nc.vector.tensor_add(
                    out=s2, in0=h4[:, :, 1, :, 0], in1=h4[:, :, 1, :, 1]
                )
                nc.vector.tensor_add(out=avg_sb[ic], in0=s1, in1=s2)
            band_fixups(avg_sb, 0, 2, 33, 31, 0, 2, 65, 63, LC)

            wl2l = wset("wl2l")
            wh2l = wset("wh2l")
            for b in range(6):
                r0 = 6 * b
                rb = min(6, OLR - r0)
                for oc in range(2):
                    ps = mmp.tile([128, 6, OLC], f32, tag="mm")
                    p = ps[:, :rb, :]
                    conv_block(
                        p,
                        [
                            (lf_sb, wl2l, r0, 0, rb, OLC),
                            (avg_sb, wh2l, r0, 0, rb, OLC),
                        ],
                        oc,
                    )
                    stg = outp.tile([128, 6, OLC], f32, tag="ostg2")
                    sg = stg.rearrange("c r w -> c (r w)")[:, : rb * OLC].rearrange(
                        "c (r w) -> c r w", w=OLC)
                    nc.scalar.activation(
                        out=sg, in_=p, func=AF.Lrelu, bias=0.0, scale=1.0,
                        alpha=SLOPE,
                    )
                    nc.sync.dma_start(
                        out=out_lf[oc * 128 : (oc + 1) * 128, r0 : r0 + rb, :],
                        in_=sg,
                    )

    if SPLIT:
        _split_multi_waits(nc, mybir)
    return nc


def _shard(inputs):
    f = lambda k: np.ascontiguousarray(np.asarray(inputs[k], dtype=np.float32))
    c_hf, c_lf, s_hf, s_lf = f("c_hf"), f("c_lf"), f("s_hf"), f("s_lf")
    xhp = np.pad(c_hf, ((0, 0), (0, 0), (4, 4), (4, 4)), mode="reflect")
    xlp = np.pad(c_lf, ((0, 0), (0, 0), (3, 3), (3, 3)), mode="reflect")
    shp = np.pad(s_hf, ((0, 0), (0, 0), (1, 1), (1, 1)), mode="reflect")
    slp = np.pad(s_lf, ((0, 0), (0, 0), (1, 1), (1, 1)), mode="reflect")

    w9 = lambda k, s=1.0: np.ascontiguousarray(
        f(k).reshape(C, C, 9).transpose(1, 2, 0) * s
    )  # [cin, tap, cout]
    wT = lambda k, s=1.0: np.ascontiguousarray(f(k).reshape(C, C).T * s)
    col = lambda k: np.ascontiguousarray(f(k).reshape(C, 1))

    shared = {
        "whsw": w9("h_sw"), "wlsw": w9("l_sw"),
        "wadah": w9("ada_h_w"), "wadal": w9("ada_l_w"),
        "wh2h": w9("h2h"), "wl2h": w9("l2h"),
        "wl2l": w9("l2l"), "wh2l": w9("h2l", 0.25),
        "w1all": np.ascontiguousarray(np.concatenate(
            [wT("h_pw", 1 / 9.0), wT("h_bw", 1 / 9.0),
             wT("l_pw", 1 / 9.0), wT("l_bw", 1 / 9.0)], axis=1)),
        "ball": np.ascontiguousarray(np.stack(
            [f(k).reshape(C) for k in ("h_sb", "h_pb", "h_bb", "ada_h_b",
                                       "l_sb", "l_pb", "l_bb", "ada_l_b")],
            axis=1)),
    }
    maps = []
    for core in range(NCORES):
        s, h = core // 2, core % 2
        m = dict(shared)
        oh = 1 - h
        m["xh"] = np.ascontiguousarray(xhp[s][:, 64 * h : 64 * h + XR, :XC])
        m["xh_ot"] = np.ascontiguousarray(c_hf[s][:, 64 * oh : 64 * oh + 64, :])
        m["xl"] = np.ascontiguousarray(xlp[s][:, 32 * h : 32 * h + XLR, :XLC])
        m["xl_ot"] = np.ascontiguousarray(c_lf[s][:, 32 * oh : 32 * oh + 32, :])
        m["sall"] = np.ascontiguousarray(np.stack([shp[s], slp[s]], axis=1))
        m["maskp"] = np.full((128, 1), float(h), np.float32)
        m["identp"] = np.eye(128, dtype=np.float32)
        maps.append(m)
    return maps


def _run(in_maps, trace=False, **kw):
    from concourse.bass_utils import run_bass_kernel_spmd

    if "nc" not in _CACHE:
        _CACHE["nc"] = _build_nc()
    return run_bass_kernel_spmd(
        _CACHE["nc"], in_maps, core_ids=list(range(NCORES)), trace=trace, **kw
    )


def kernel(**inputs):
    res = _run(_shard(inputs))
    hf = np.zeros((B, C, 128, 128), np.float32)
    lf = np.zeros((B, C, 64, 64), np.float32)
    for core in range(NCORES):
        s, h = core // 2, core % 2
        hf[s][:, 64 * h : 64 * h + OR_, :] = res.results[core]["out_hf"]
        lf[s][:, 32 * h : 32 * h + OLR, :] = res.results[core]["out_lf"]
    return hf, lf


# revision 36
# speedup vs baseline: 1.1611x; 1.0101x over previous
# AdaOctConv distributed Trainium2 kernel (8 NeuronCores, SPMD, no collectives).
#
# Sharding: 4 samples x 2 spatial halves = 8 cores. Each core computes a
# 64-row band of hf_out and a 32-row band of lf_out of one sample. Internal
# split halos come in via host-side overlapped shards; image-border halos are
# reflect-copies of computed interior rows, applied on device as mask-blended
# row/col fixups so all 8 cores share one SPMD graph.
#
# Math folding (per channel c, per sample):
#   instance_norm -> depthwise3x3(per-sample w) -> *w_point + bias  ==
#   depthwise3x3(raw x; ws*(wp*rsqrt(var+eps))) + beta, with beta folded into
#   the following dense conv's bias. The heavy work is six 3x3 dense convs,
#   each computed as 9 shift-matmuls (bf16 in, fp32 PSUM accumulation).
import sys

for _p in ("/opt/trn_rl_repo",):
    if _p not in sys.path:
        sys.path.append(_p)

import os

import numpy as np

B, C = 4, 256
EPS, SLOPE = 1e-5, 0.01
NCORES = 8

XR, XC = 72, 136      # x band (reflect-padded +-4)
ZR, ZC = 70, 134      # depthwise output band
HR, HC = 68, 132      # hf band
OR_, OC_ = 64, 128    # hf output band
XLR, XLC = 38, 70
ZLR, ZLC = 36, 68
LR, LC = 34, 66
OLR, OLC = 32, 64
UR, UC = 66, 130      # upsampled-lf band
UPT = 10              # rows per up tile (8 tiles of [8g, 8g+10))

_CACHE = {}


def _split_multi_waits(nc, mybir):
    """This walrus build accepts at most one sync-wait per instruction;
    hoist extra waits onto single-wait NOPs on the same engine."""
    for fn in nc.m.functions:
        for bb in fn.blocks:
            new = []
            for ins in bb.instructions:
                si = ins.sync_info
                if si and si.on_wait and len(si.on_wait) > 1:
                    waits = list(si.on_wait)
                    for w in waits[:-1]:
                        nop = mybir.InstNoOp(
                            name=f"I-ws-{nc.next_id()}", ins=[], outs=[]
                        )
                        nop.engine = ins.engine
                        nop.sync_info = mybir.SyncInfo(on_wait=[w], on_update=[])
                        new.append(nop)
                    ins.sync_info = mybir.SyncInfo(
                        on_wait=[waits[-1]], on_update=list(si.on_update or [])
                    )
                new.append(ins)
            bb.instructions[:] = new


def _build_nc():
    import contextlib

    import concourse.bass as bass
    import concourse.mybir as mybir
    import concourse.tile as tile

    f32 = mybir.dt.float32
    bf16 = mybir.dt.bfloat16
    AF = mybir.ActivationFunctionType
    OP = mybir.AluOpType
    AX = mybir.AxisListType

    STAGES = int(os.environ.get("ADAOCT_STAGES", "9"))
    SPLIT = os.environ.get("ADAOCT_SPLIT", "1") == "1"

    nc = bass.Bass()

    def par(name, shape, out=False):
        return nc.declare_dram_parameter(name, list(shape), f32, isOutput=out)

    xh = par("xh", (C, XR, XC))
    W1N = ("whpw", "whbw", "wlpw", "wlbw")
    BN = ("hsb", "hpb", "hbb", "ahb", "lsb", "lpb", "lbb", "alb")
    xh_ot = par("xh_ot", (C, 64, 128))
    xl = par("xl", (C, XLR, XLC))
    xl_ot = par("xl_ot", (C, 32, 64))
    identp = par("identp", (128, 128))
    maskp = par("maskp", (128, 1))
    wsets = {
        n: par(n, (C, 9, C))
        for n in ("whsw", "wlsw", "wadal", "wadah", "wh2h", "wl2h", "wl2l", "wh2l")
    }
    w1all = par("w1all", (C, 4 * C))  # whpw|whbw|wlpw|wlbw along cols
    ball = par("ball", (C, 8))  # hsb hpb hbb ahb lsb lpb lbb alb
    sall = par("sall", (C, 2, 5, 5))  # sh | sl
    out_hf = par("out_hf", (C, OR_, OC_), out=True)
    out_lf = par("out_lf", (C, OLR, OLC), out=True)

    TAPS = [(dy, dx) for dy in range(3) for dx in range(3)]

    class _StopBuild(Exception):
        pass

    with tile.TileContext(nc) as tc:
        es = contextlib.ExitStack()
        with es, contextlib.suppress(_StopBuild):
            persist = es.enter_context(tc.tile_pool(name="persist", bufs=1))
            small = es.enter_context(tc.tile_pool(name="small", bufs=1))
            wpool = es.enter_context(tc.tile_pool(name="wpool", bufs=4))
            wstg = es.enter_context(tc.tile_pool(name="wstg", bufs=2))
            psmall = es.enter_context(tc.tile_pool(name="psmall", bufs=2, space="PSUM"))
            mmp = es.enter_context(tc.tile_pool(name="mmp", bufs=6, space="PSUM"))
            accp = es.enter_context(tc.tile_pool(name="accp", bufs=1))
            z2p = es.enter_context(tc.tile_pool(name="z2p", bufs=4))
            upp = es.enter_context(tc.tile_pool(name="upp", bufs=2))
            outp = es.enter_context(tc.tile_pool(name="outp", bufs=2))
            fixp = es.enter_context(tc.tile_pool(name="fixp", bufs=1))

            # ---------- persistent tensors ----------
            xh_sb = [persist.tile([128, XR, XC], bf16, tag=f"xh{i}") for i in range(2)]
            xl_sb = [persist.tile([128, XLR, XLC], bf16, tag=f"xl{i}") for i in range(2)]
            hf_sb = [persist.tile([128, HR, HC], bf16, tag=f"hf{i}") for i in range(2)]
            lf_sb = [persist.tile([128, LR, LC], bf16, tag=f"lf{i}") for i in range(2)]
            avg_sb = [persist.tile([128, LR, LC], bf16, tag=f"av{i}") for i in range(2)]

            eps_t = small.tile([128, 1], f32, tag="eps")
            nc.vector.memset(eps_t, EPS)
            mask_t = small.tile([128, 1], f32, tag="mask")
            nc.sync.dma_start(out=mask_t, in_=maskp[:, :])
            imask_t = small.tile([128, 1], f32, tag="imask")
            nc.scalar.activation(
                out=imask_t, in_=mask_t, func=AF.Copy, bias=1.0, scale=-1.0
            )

            identf = small.tile([128, 128], f32, tag="identf", name="identf")
            nc.sync.dma_start(out=identf, in_=identp[:, :])
            ident_b = small.tile([128, 128], bf16, tag="identb", name="identb")
            nc.scalar.copy(out=ident_b, in_=identf)

            def blend(cur, ref, sel, cols):
                """cur = cur + sel*(ref - cur) over a [128, 1, cols] slice."""
                d = fixp.tile([128, 1, 144], f32, tag="fixd")
                dd = d[:, :, :cols]
                nc.vector.tensor_sub(out=dd, in0=ref, in1=cur)
                nc.vector.scalar_tensor_tensor(
                    out=cur, in0=dd, scalar=sel, in1=cur, op0=OP.mult, op1=OP.add
                )

            bias_sb = {}
            for i in range(2):
                bt = small.tile([128, 8], f32, tag=f"ball{i}", name=f"ball{i}")
                nc.sync.dma_start(out=bt, in_=ball[i * 128 : (i + 1) * 128, :])
                for j, n in enumerate(BN):
                    bias_sb[(n, i)] = bt[:, j : j + 1]

            w1_sb = {}
            for i in range(2):
                stg = wstg.tile([128, 1152], f32, tag="wstg", name="w1stg")
                nc.sync.dma_start(
                    out=stg[:, : 4 * C].rearrange("c (k o) -> c k o", k=4),
                    in_=w1all[i * 128 : (i + 1) * 128, :].rearrange(
                        "c (k o) -> c k o", k=4),
                )
                t = small.tile([128, 4, C], bf16, tag=f"w1_{i}", name=f"w1_{i}")
                nc.scalar.copy(out=t, in_=stg[:, : 4 * C].rearrange(
                    "c (k o) -> c k o", k=4))
                for j, n in enumerate(W1N):
                    w1_sb[(n, i)] = t[:, j, :]

            sbuf_s = {}
            for i in range(2):
                tf = small.tile([128, 2, 5, 5], f32, tag=f"sf{i}", name=f"sf{i}")
                nc.sync.dma_start(out=tf, in_=sall[i * 128 : (i + 1) * 128, :, :, :])
                tb = small.tile([128, 2, 5, 5], bf16, tag=f"sb{i}", name=f"sb{i}")
                nc.scalar.copy(out=tb, in_=tf)
                for j, nm in enumerate(("sh", "sl")):
                    sbuf_s[(nm, i, "f")] = tf[:, j]
                    sbuf_s[(nm, i, "b")] = tb[:, j]

            # ---------- weight set loading ----------
            wcache = {}

            def wset(name):
                if name in wcache:
                    return wcache[name]
                p = wsets[name]
                tiles = []
                for i in range(2):
                    t = wpool.tile([128, 9, 2, 128], bf16, tag="wset")
                    flat = t.rearrange("c t o q -> c (t o q)")
                    src = p[i * 128 : (i + 1) * 128, :, :].rearrange("c t o -> c (t o)")
                    for j in range(2):
                        stg = wstg.tile([128, 1152], f32, tag="wstg")
                        nc.sync.dma_start(
                            out=stg, in_=src[:, j * 1152 : (j + 1) * 1152]
                        )
                        nc.scalar.copy(out=flat[:, j * 1152 : (j + 1) * 1152], in_=stg)
                    tiles.append(t)
                wcache[name] = tiles
                return tiles

            # ---------- instance-norm stats ----------
            # mean/var over [own band interior (bf16, in SBUF)] + [other
            # half's rows, streamed f32]. Sums via ScalarE accum_out.
            def stats_stream(other, hw, bpool):
                npc = (hw // 2) // 1024
                bn_t = []
                for i in range(2):
                    nm = f"{other.name}{i}"
                    bnst = small.tile([128, 2 * npc, 6], f32, tag=f"bn_{nm}",
                                      name=f"bn_{nm}")
                    flat = other[i * 128 : (i + 1) * 128, :, :].rearrange(
                        "c h w -> c (h w)"
                    )
                    for k in range(npc):
                        pc = bpool.tile([128, 1024], f32, tag="xstg2", name="xstg2")
                        nc.gpsimd.dma_start(
                            out=pc, in_=flat[:, k * 1024 : (k + 1) * 1024]
                        )
                        for j in range(2):
                            nc.vector.bn_stats(
                                out=bnst[:, 2 * k + j, :],
                                in_=pc[:, j * 512 : (j + 1) * 512],
                            )
                    bn_t.append(bnst)
                return bn_t

            def stats(bn_t, band_sb, bint, hw, bpool):
                r0, r1, c0, c1 = bint
                half = hw // 2
                m_t, inv_t = [], []
                for i in range(2):
                    nm = bn_t[i].tensor.name
                    sums = small.tile([128, 5, 2], f32, tag=f"s5_{nm}",
                                      name=f"s5_{nm}")
                    mvs = small.tile([128, 2], f32, tag=f"mvs_{nm}",
                                     name=f"mvs_{nm}")
                    nc.vector.bn_aggr(out=mvs, in_=bn_t[i])
                    # stream half back to (sum, sumsq) space
                    nc.vector.tensor_scalar_mul(out=sums[:, 4, 0:1],
                                                in0=mvs[:, 0:1], scalar1=float(half))
                    msq = small.tile([128, 1], f32, tag=f"msq_{nm}",
                                     name=f"msq_{nm}")
                    nc.vector.tensor_mul(out=msq, in0=mvs[:, 0:1], in1=mvs[:, 0:1])
                    nc.vector.tensor_add(out=msq, in0=msq, in1=mvs[:, 1:2])
                    nc.vector.tensor_scalar_mul(out=sums[:, 4, 1:2], in0=msq,
                                                scalar1=float(half))
                    rows = r1 - r0
                    q = rows // 4
                    for j in range(4):
                        seg = band_sb[i][:, r0 + j * q : r0 + (j + 1) * q, c0:c1]
                        scr = bpool.tile([128, 2048], bf16, tag="scr", name="scr",
                                         bufs=1)
                        sg = scr[:, : seg.free_size()]
                        nc.scalar.activation(out=sg, in_=seg, func=AF.Copy,
                                             accum_out=sums[:, j, 0:1])
                        nc.scalar.activation(out=sg, in_=seg, func=AF.Square,
                                             accum_out=sums[:, j, 1:2])
                    mv = small.tile([128, 2], f32, tag=f"mv_{nm}", name=f"mv_{nm}")
                    nc.vector.tensor_reduce(
                        out=mv, in_=sums.rearrange("c n k -> c k n"),
                        axis=AX.X, op=OP.add)
                    m = small.tile([128, 1], f32, tag=f"m_{nm}", name=f"m_{nm}")
                    nc.vector.tensor_scalar_mul(out=m, in0=mv[:, 0:1],
                                                scalar1=1.0 / hw)
                    var = small.tile([128, 1], f32, tag=f"v_{nm}", name=f"v_{nm}")
                    nc.vector.tensor_scalar_mul(out=var, in0=mv[:, 1:2],
                                                scalar1=1.0 / hw)
                    msqt = small.tile([128, 1], f32, tag=f"mq_{nm}", name=f"mq_{nm}")
                    nc.vector.tensor_mul(out=msqt, in0=m, in1=m)
                    nc.vector.tensor_sub(out=var, in0=var, in1=msqt)
                    inv = small.tile([128, 1], f32, tag=f"i_{nm}", name=f"i_{nm}")
                    nc.scalar.activation(
                        out=inv, in_=var, func=AF.Sqrt, bias=eps_t, scale=1.0
                    )
                    nc.vector.reciprocal(out=inv, in_=inv)
                    m_t.append(m)
                    inv_t.append(inv)
                return m_t, inv_t

            def load_band(param, dst, rows, cols, bpool, rchunk):
                for i in range(2):
                    src = param[i * 128 : (i + 1) * 128, :, :]
                    for r0 in range(0, rows, rchunk):
                        r1 = min(r0 + rchunk, rows)
                        stg = bpool.tile([128, 3264], f32, tag="xstg")
                        s3 = stg[:, : (r1 - r0) * cols].rearrange(
                            "c (r w) -> c r w", w=cols
                        )
                        nc.gpsimd.dma_start(out=s3, in_=src[:, r0:r1, :])
                        nc.scalar.copy(out=dst[i][:, r0:r1, :], in_=s3)

            bpool = es.enter_context(tc.tile_pool(name="bpool", bufs=2))
            wset("wlsw")
            wset("wadal")
            sums_l = stats_stream(xl_ot, 64 * 64, bpool)
            load_band(xl, xl_sb, XLR, XLC, bpool, 14)
            m_l, inv_l = stats(sums_l, xl_sb, (3, 35, 3, 67), 64 * 64, bpool)

            # ---------- kernel_predict + folds ----------
            # ws2 = ws*wp (stats-free; inv is folded into the dense-conv
            # stationary later). beta = b - (wp*inv)*m*S.
            def kp_A(skey, wsw_name, w1pw, w1bw, bsb, bpb, bbb, tag, want_diag=False):
                wsw = wset(wsw_name)
                s_f = [sbuf_s[(skey, i, "f")] for i in range(2)]
                s_b = [sbuf_s[(skey, i, "b")] for i in range(2)]
                pooled_b = []
                for i in range(2):
                    pf = small.tile([128, 1], f32, tag=f"poo{tag}{i}", name=f"poo{tag}{i}")
                    nc.vector.reduce_sum(out=pf, in_=s_f[i][:, 1:4, 1:4], axis=AX.XY)
                    pb = small.tile([128, 1], bf16, tag=f"poob{tag}{i}", name=f"poob{tag}{i}")
                    nc.scalar.copy(out=pb, in_=pf)
                    pooled_b.append(pb)

                ws2, wp_t, bv_t, S_t, diag = [], [], [], [], []
                for oc in range(2):
                    ps = psmall.tile([128, 9], f32, tag="ps9", name="ps9")
                    ps3 = ps.rearrange("c (a b) -> c a b", a=3)
                    n = 0
                    for t, (dy, dx) in enumerate(TAPS):
                        for ic in range(2):
                            nc.tensor.matmul(
                                ps3,
                                wsw[ic][:, t, oc, :],
                                s_b[ic][:, dy : dy + 3, dx : dx + 3],
                                start=(n == 0),
                                stop=(n == 17),
                            )
                            n += 1
                    wf = small.tile([128, 9], f32, tag=f"ws{tag}{oc}", name=f"ws{tag}{oc}")
                    nc.scalar.activation(out=wf, in_=ps, func=AF.Identity,
                                         bias=bias_sb[(bsb, oc)], scale=1.0)

                    ps1 = psmall.tile([128, 9], f32, tag="ps9", name="ps9")
                    for ic in range(2):
                        nc.tensor.matmul(
                            ps1[:, 0:1],
                            w1_sb[(w1pw, ic)][:, oc * 128 : (oc + 1) * 128],
                            pooled_b[ic],
                            start=(ic == 0),
                            stop=(ic == 1),
                        )
                    wp = small.tile([128, 1], f32, tag=f"wp{tag}{oc}", name=f"wp{tag}{oc}")
                    nc.scalar.activation(out=wp, in_=ps1[:, 0:1], func=AF.Identity,
                                         bias=bias_sb[(bpb, oc)], scale=1.0)
                    wp_t.append(wp)

                    ps2 = psmall.tile([128, 9], f32, tag="ps9", name="ps9")
                    for ic in range(2):
                        nc.tensor.matmul(
                            ps2[:, 0:1],
                            w1_sb[(w1bw, ic)][:, oc * 128 : (oc + 1) * 128],
                            pooled_b[ic],
                            start=(ic == 0),
                            stop=(ic == 1),
                        )
                    bv = small.tile([128, 1], f32, tag=f"bv{tag}{oc}", name=f"bv{tag}{oc}")
                    nc.scalar.activation(out=bv, in_=ps2[:, 0:1], func=AF.Identity,
                                         bias=bias_sb[(bbb, oc)], scale=1.0)
                    bv_t.append(bv)

                    w2 = small.tile([128, 9], f32, tag=f"w2{tag}{oc}", name=f"w2{tag}{oc}")
                    nc.scalar.activation(out=w2, in_=wf, func=AF.Identity,
                                         bias=0.0, scale=wp)
                    ws2.append(w2)
                    s_s = small.tile([128, 1], f32, tag=f"S{tag}{oc}", name=f"S{tag}{oc}")
                    nc.vector.reduce_sum(out=s_s, in_=wf, axis=AX.X)
                    S_t.append(s_s)
                    if want_diag:
                        dg = small.tile([128, 9, 128], bf16, tag=f"dg{tag}{oc}", name=f"dg{tag}{oc}")
                        for t in range(9):
                            nc.scalar.activation(
                                out=dg[:, t, :], in_=ident_b, func=AF.Identity,
                                bias=0.0, scale=w2[:, t : t + 1])
                        diag.append(dg)
                return ws2, diag, wp_t, bv_t, S_t

            def kp_B(wada_name, bada, wp_t, bv_t, S_t, m_t, inv_t, tag):
                """bias2 from unscaled wada, then scale wada in place by
                inv[cin] (per-partition)."""
                wada = wset(wada_name)
                beta_b = []
                for oc in range(2):
                    tmp = small.tile([128, 1], f32, tag=f"t1{tag}{oc}", name=f"t1{tag}{oc}")
                    nc.scalar.activation(out=tmp, in_=wp_t[oc], func=AF.Identity,
                                         bias=0.0, scale=inv_t[oc])
                    nc.scalar.activation(out=tmp, in_=tmp, func=AF.Identity,
                                         bias=0.0, scale=m_t[oc])
                    nc.scalar.activation(out=tmp, in_=tmp, func=AF.Identity,
                                         bias=0.0, scale=S_t[oc])
                    nc.scalar.mul(out=tmp, in_=tmp, mul=-1.0)
                    bb_ = small.tile([128, 1], bf16, tag=f"beb{tag}{oc}", name=f"beb{tag}{oc}")
                    nc.scalar.activation(out=bb_, in_=bv_t[oc], func=AF.Identity,
                                         bias=tmp, scale=1.0)
                    beta_b.append(bb_)
                bias2 = []
                for oc in range(2):
                    ps = psmall.tile([128, 9], f32, tag="ps9", name="ps9")
                    n = 0
                    for t in range(9):
                        for ic in range(2):
                            nc.tensor.matmul(
                                ps[:, 0:1],
                                wada[ic][:, t, oc, :],
                                beta_b[ic],
                                start=(n == 0),
                                stop=(n == 17),
                            )
                            n += 1
                    b2 = small.tile([128, 1], f32, tag=f"b2{tag}{oc}", name=f"b2{tag}{oc}")
                    nc.scalar.activation(out=b2, in_=ps[:, 0:1], func=AF.Identity,
                                         bias=bias_sb[(bada, oc)], scale=1.0)
                    bias2.append(b2)
                for ic in range(2):
                    wf_ = wada[ic].rearrange("c t o q -> c (t o q)")
                    nc.scalar.activation(out=wf_, in_=wf_, func=AF.Identity,
                                         bias=0.0, scale=inv_t[ic])
                return bias2, wada

            # ---------- depthwise (9-tap MAC on the Vector engine) ----------
            def dw_tile(x_sb, ws2, z0, zrows, W, tag, fix):
                """z[c, q, w] = sum_t ws2[c,t] * x[c, q+dy, w+dx]; cols
                [0, W-2) valid. Returns [128, zrows, W] bf16 views."""
                topq, tops, botq, bots, ctop, cts, cbot, cbs = fix
                zc = W - 2
                out = []
                for ic in range(2):
                    acc = accp.tile([128, 14, 134], bf16, tag="acc", name="acc")
                    a = acc[:, :zrows, :zc]
                    zt = z2p.tile([128, 14, zc], bf16, tag=f"z2{tag}", name=f"z2{tag}")
                    z = zt[:, :zrows, :]
                    zv = z
                    for t, (dy, dx) in enumerate(TAPS):
                        in0 = x_sb[ic][:, z0 + dy : z0 + dy + zrows, dx : dx + zc]
                        sc = ws2[ic][:, t : t + 1]
                        if t == 0:
                            nc.vector.tensor_scalar_mul(out=a, in0=in0, scalar1=sc)
                        elif t < 8:
                            nc.vector.scalar_tensor_tensor(
                                out=a, in0=in0, scalar=sc, in1=a,
                                op0=OP.mult, op1=OP.add,
                            )
                        else:
                            nc.vector.scalar_tensor_tensor(
                                out=zv, in0=in0, scalar=sc, in1=a,
                                op0=OP.mult, op1=OP.add,
                            )
                    if z0 <= topq < z0 + zrows:
                        blend(z[:, topq - z0 : topq - z0 + 1, :zc],
                              z[:, tops - z0 : tops - z0 + 1, :zc], imask_t, zc)
                    if z0 <= botq < z0 + zrows:
                        blend(z[:, botq - z0 : botq - z0 + 1, :zc],
                              z[:, bots - z0 : bots - z0 + 1, :zc], mask_t, zc)
                    nc.vector.tensor_copy(
                        out=z[:, :, ctop : ctop + 1], in_=z[:, :, cts : cts + 1]
                    )
                    nc.vector.tensor_copy(
                        out=z[:, :, cbot : cbot + 1], in_=z[:, :, cbs : cbs + 1]
                    )
                    out.append(z)
                return out

            def dw_tile_pe(x_sb, diag, z0, zrows, W, tag, fix):
                """Same contract as dw_tile, but computed on the TensorEngine
                as 9 accumulated diag-matmuls over flat 512-col windows."""
                topq, tops, botq, bots, ctop, cts, cbot, cbs = fix
                L = zrows * W - 2
                out = []
                for ic in range(2):
                    zt = z2p.tile([128, 14, W], bf16, tag=f"z2{tag}", name=f"z2p{tag}")
                    zflat = zt.rearrange("c r w -> c (r w)")
                    xflat = x_sb[ic].rearrange("c r w -> c (r w)")
                    for off in range(0, L, 512):
                        n = min(512, L - off)
                        ps = mmp.tile([128, 512], f32, tag="mm", name="zps")
                        for t, (dy, dx) in enumerate(TAPS):
                            base = (z0 + dy) * W + dx + off
                            nc.tensor.matmul(
                                ps[:, :n],
                                diag[ic][:, t, :],
                                xflat[:, base : base + n],
                                start=(t == 0),
                                stop=(t == 8),
                            )
                        nc.scalar.copy(out=zflat[:, off : off + n], in_=ps[:, :n])
                    z = zt[:, :zrows, :]
                    if z0 <= topq < z0 + zrows:
                        blend(z[:, topq - z0 : topq - z0 + 1, : W - 2],
                              z[:, tops - z0 : tops - z0 + 1, : W - 2], imask_t, W - 2)
                    if z0 <= botq < z0 + zrows:
                        blend(z[:, botq - z0 : botq - z0 + 1, : W - 2],
                              z[:, bots - z0 : bots - z0 + 1, : W - 2], mask_t, W - 2)
                    nc.vector.tensor_copy(
                        out=z[:, :, ctop : ctop + 1], in_=z[:, :, cts : cts + 1]
                    )
                    nc.vector.tensor_copy(
                        out=z[:, :, cbot : cbot + 1], in_=z[:, :, cbs : cbs + 1]
                    )
                    out.append(z)
                return out

            def conv_block(psum, srcs, oc):
                n = 0
                total = 18 * len(srcs)
                for tiles, wt, r0, c0, rows, cols in srcs:
                    for t, (dy, dx) in enumerate(TAPS):
                        for ic in range(2):
                            nc.tensor.matmul(
                                psum,
                                wt[ic][:, t, oc, :],
                                tiles[ic][:, r0 + dy : r0 + dy + rows,
                                          c0 + dx : c0 + dx + cols],
                                start=(n == 0),
                                stop=(n == total - 1),
                            )
                            n += 1

            def band_fixups(sb, topq, tops, botq, bots, ctop, cts, cbot, cbs, cols):
                for ic in range(2):
                    blend(sb[ic][:, topq : topq + 1, :],
                          sb[ic][:, tops : tops + 1, :], imask_t, cols)
                    blend(sb[ic][:, botq : botq + 1, :],
                          sb[ic][:, bots : bots + 1, :], mask_t, cols)
                    nc.vector.tensor_copy(
                        out=sb[ic][:, :, ctop : ctop + 1],
                        in_=sb[ic][:, :, cts : cts + 1],
                    )
                    nc.vector.tensor_copy(
                        out=sb[ic][:, :, cbot : cbot + 1],
                        in_=sb[ic][:, :, cbs : cbs + 1],
                    )

            # ================= LF branch =================
            if STAGES < 2:
                dbg = persist.tile([128, 2048], f32, tag="dbg", name="dbg")
                nc.vector.memset(dbg, 0.0)
                d3 = dbg.rearrange("c (r w) -> c r w", w=OC_)
                for oc in range(2):
                    for r0 in range(0, OR_, 16):
                        nc.sync.dma_start(
                            out=out_hf[oc * 128 : (oc + 1) * 128, r0 : r0 + 16, :],
                            in_=d3)
                d4 = dbg[:, : 16 * OLC].rearrange("c (r w) -> c r w", w=OLC)
                for oc in range(2):
                    for r0 in range(0, OLR, 16):
                        nc.sync.dma_start(
                            out=out_lf[oc * 128 : (oc + 1) * 128, r0 : r0 + 16, :],
                            in_=d4)
                raise _StopBuild()
            ws2_l, diag_l, wp_l, bv_l, S_l = kp_A(
                "sl", "wlsw", "wlpw", "wlbw", "lsb", "lpb", "lbb", "l"
            )
            bias2_l, wadal = kp_B("wadal", "alb", wp_l, bv_l, S_l, m_l, inv_l, "l")

            # hf stats stream first (inv_h gates the ada_h stationary),
            # then the hf band
            sums_h = stats_stream(xh_ot, 128 * 128, bpool)
            load_band(xh, xh_sb, XR, XC, bpool, 7)
            m_h, inv_h = stats(sums_h, xh_sb, (4, 68, 4, 132), 128 * 128, bpool)
            ws2_h, diag_h, wp_h, bv_h, S_h = kp_A(
                "sh", "whsw", "whpw", "whbw", "hsb", "hpb", "hbb", "h",
                want_diag=True,
            )

            if STAGES < 3:
                raise _StopBuild()
            zl_fix = (1, 3, 34, 32, 1, 3, 66, 64)
            for t0 in range(3):
                z0 = 12 * t0
                zt = dw_tile(xl_sb, ws2_l, z0, min(14, ZLR - z0), XLC, "l", zl_fix)
                if STAGES < 4:
                    continue
                for b in range(2 * t0, min(2 * t0 + 2, 6)):
                    r0 = 6 * b
                    rb = min(6, LR - r0)
                    q0 = r0 - 12 * t0
                    for oc in range(2):
                        ps = mmp.tile([128, 6, LC], f32, tag="mm", name="mm")
                        p = ps[:, :rb, :]
                        conv_block(p, [(zt, wadal, q0, 0, rb, LC)], oc)
                        nc.scalar.activation(
                            out=lf_sb[oc][:, r0 : r0 + rb, :], in_=p,
                            func=AF.Lrelu, bias=bias2_l[oc], scale=1.0, alpha=SLOPE,
                        )
            if STAGES < 4:
                raise _StopBuild()
            band_fixups(lf_sb, 0, 2, 33, 31, 0, 2, 65, 63, LC)

            # ================= HF branch =================
            bias2_h, wadah = kp_B("wadah", "ahb", wp_h, bv_h, S_h, m_h, inv_h, "h")

            zh_fix = (2, 4, 67, 65, 2, 4, 131, 129)
            for t0 in range(6):
                z0 = 12 * t0
                zrows = min(14, ZR - z0)
                if t0 in (0, 2, 4):
                    zt = dw_tile_pe(xh_sb, diag_h, z0, zrows, XC, "h", zh_fix)
                else:
                    zt = dw_tile(xh_sb, ws2_h, z0, zrows, XC, "h", zh_fix)
                for b in range(4 * t0, min(4 * t0 + 4, 23)):
                    r0 = 3 * b
                    rb = min(3, HR - r0)
                    q0 = r0 - 12 * t0
                    for oc in range(2):
                        ps = mmp.tile([128, 3, HC], f32, tag="mm", name="mm")
                        p = ps[:, :rb, :]
                        conv_block(p, [(zt, wadah, q0, 0, rb, HC)], oc)
                        nc.scalar.activation(
                            out=hf_sb[oc][:, r0 : r0 + rb, :], in_=p,
                            func=AF.Lrelu, bias=bias2_h[oc], scale=1.0, alpha=SLOPE,
                        )
            band_fixups(hf_sb, 1, 3, 66, 64, 1, 3, 130, 128, HC)

            # ================= cross-frequency fusion =================
            if STAGES < 5:
                raise _StopBuild()
            wh2h = wset("wh2h")
            wl2h = wset("wl2h")

            # special up rows: u=0 -> lf[1]+mask*(lf[0]-lf[1]);
            #                  u=65 -> lf[33]+mask*(lf[32]-lf[33])
            sprow = {}
            for key, ja, jb in (("r0", 1, 0), ("r65", 33, 32)):
                rows = []
                for ic in range(2):
                    d = small.tile([128, 1, LC], f32, tag=f"upd{key}{ic}")
                    nc.vector.tensor_sub(
                        out=d, in0=lf_sb[ic][:, jb : jb + 1, :],
                        in1=lf_sb[ic][:, ja : ja + 1, :],
                    )
                    r = small.tile([128, 1, LC], bf16, tag=f"upr{key}{ic}")
                    nc.vector.scalar_tensor_tensor(
                        out=r, in0=d, scalar=mask_t,
                        in1=lf_sb[ic][:, ja : ja + 1, :],
                        op0=OP.mult, op1=OP.add,
                    )
                    rows.append(r)
                sprow[key] = rows

            def up_cols(dst_rows, src_rows):
                # dst [128, n, 130] <- src [128, n, 64] column-doubling w/ edges
                nc.vector.tensor_copy(out=dst_rows[:, :, 1:129:2], in_=src_rows)
                nc.vector.tensor_copy(out=dst_rows[:, :, 2:130:2], in_=src_rows)
                nc.vector.tensor_copy(
                    out=dst_rows[:, :, 0:1], in_=src_rows[:, :, 0:1]
                )
                nc.vector.tensor_copy(
                    out=dst_rows[:, :, 129:130], in_=src_rows[:, :, 63:64]
                )

            def build_up_tile(g):
                tiles = []
                u0 = 8 * g
                for ic in range(2):
                    ut = upp.tile([128, UPT, UC], bf16, tag="up")
                    ev = [i for i in range(0, UPT, 2) if not (g == 0 and i == 0)]
                    od = [i for i in range(1, UPT, 2) if not (g == 7 and i == 9)]
                    for phase in (ev, od):
                        i0, cnt = phase[0], len(phase)
                        j0 = (u0 + i0 - 1) // 2 + 1
                        dst = ut[:, i0 : i0 + 2 * cnt - 1 : 2, :]
                        src = lf_sb[ic][:, j0 : j0 + cnt, 1:65]
                        up_cols(dst, src)
                    if g == 0:
                        up_cols(ut[:, 0:1, :], sprow["r0"][ic][:, :, 1:65])
                    if g == 7:
                        up_cols(ut[:, 9:10, :], sprow["r65"][ic][:, :, 1:65])
                    tiles.append(ut)
                return tiles

            if STAGES < 6:
                for g in range(8):
                    build_up_tile(g)
                raise _StopBuild()
            up_tiles = {}
            for r in range(16):
                g = r // 2
                if g not in up_tiles:
                    up_tiles[g] = build_up_tile(g)
                u_local = 4 * r - 8 * g
                for oc in range(2):
                    ps = mmp.tile([128, 4, OC_], f32, tag="mm")
                    conv_block(
                        ps,
                        [
                            (hf_sb, wh2h, 4 * r + 1, 1, 4, OC_),
                            (up_tiles[g], wl2h, u_local, 0, 4, OC_),
                        ],
                        oc,
                    )
                    stg = outp.tile([128, 4, OC_], f32, tag="ostg")
                    nc.scalar.activation(
                        out=stg, in_=ps, func=AF.Lrelu, bias=0.0, scale=1.0,
                        alpha=SLOPE,
                    )
                    nc.sync.dma_start(
                        out=out_hf[oc * 128 : (oc + 1) * 128, 4 * r : 4 * r + 4, :],
                        in_=stg,
                    )

            # avgpool of hf (0.25 folded into h2l weights host-side)
            for ic in range(2):
                h4 = hf_sb[ic].rearrange("c (r p) (w q) -> c r p w q", p=2, q=2)
                s1 = accp.tile([128, LR, LC], bf16, tag="avt")
                nc.vector.tensor_add(
                    out=s1, in0=h4[:, :, 0, :, 0], in1=h4[:, :, 0, :, 1]
                )
                s2 = accp.tile([128, LR, LC], bf16, tag="avt")
                nc.vector.tensor_add(
                    out=s2, in0=h4[:, :, 1, :, 0], in1=h4[:, :, 1, :, 1]
                )
                nc.vector.tensor_add(out=avg_sb[ic], in0=s1, in1=s2)
            band_fixups(avg_sb, 0, 2, 33, 31, 0, 2, 65, 63, LC)

            wl2l = wset("wl2l")
            wh2l = wset("wh2l")
            for b in range(6):
                r0 = 6 * b
                rb = min(6, OLR - r0)
                for oc in range(2):
                    ps = mmp.tile([128, 6, OLC], f32, tag="mm")
                    p = ps[:, :rb, :]
                    conv_block(
                        p,
                        [
                            (lf_sb, wl2l, r0, 0, rb, OLC),
                            (avg_sb, wh2l, r0, 0, rb, OLC),
                        ],
                        oc,
                    )
                    stg = outp.tile([128, 6, OLC], f32, tag="ostg2")
                    sg = stg.rearrange("c r w -> c (r w)")[:, : rb * OLC].rearrange(
                        "c (r w) -> c r w", w=OLC)
                    nc.scalar.activation(
                        out=sg, in_=p, func=AF.Lrelu, bias=0.0, scale=1.0,
                        alpha=SLOPE,
                    )
                    nc.sync.dma_start(
                        out=out_lf[oc * 128 : (oc + 1) * 128, r0 : r0 + rb, :],
                        in_=sg,
                    )

    if SPLIT:
        _split_multi_waits(nc, mybir)
    return nc


def _shard(inputs):
    f = lambda k: np.ascontiguousarray(np.asarray(inputs[k], dtype=np.float32))
    c_hf, c_lf, s_hf, s_lf = f("c_hf"), f("c_lf"), f("s_hf"), f("s_lf")
    xhp = np.pad(c_hf, ((0, 0), (0, 0), (4, 4), (4, 4)), mode="reflect")
    xlp = np.pad(c_lf, ((0, 0), (0, 0), (3, 3), (3, 3)), mode="reflect")
    shp = np.pad(s_hf, ((0, 0), (0, 0), (1, 1), (1, 1)), mode="reflect")
    slp = np.pad(s_lf, ((0, 0), (0, 0), (1, 1), (1, 1)), mode="reflect")

    w9 = lambda k, s=1.0: np.ascontiguousarray(
        f(k).reshape(C, C, 9).transpose(1, 2, 0) * s
    )  # [cin, tap, cout]
    wT = lambda k, s=1.0: np.ascontiguousarray(f(k).reshape(C, C).T * s)
    col = lambda k: np.ascontiguousarray(f(k).reshape(C, 1))

    shared = {
        "whsw": w9("h_sw"), "wlsw": w9("l_sw"),
        "wadah": w9("ada_h_w"), "wadal": w9("ada_l_w"),
        "wh2h": w9("h2h"), "wl2h": w9("l2h"),
        "wl2l": w9("l2l"), "wh2l": w9("h2l", 0.25),
        "w1all": np.ascontiguousarray(np.concatenate(
            [wT("h_pw", 1 / 9.0), wT("h_bw", 1 / 9.0),
             wT("l_pw", 1 / 9.0), wT("l_bw", 1 / 9.0)], axis=1)),
        "ball": np.ascontiguousarray(np.stack(
            [f(k).reshape(C) for k in ("h_sb", "h_pb", "h_bb", "ada_h_b",
                                       "l_sb", "l_pb", "l_bb", "ada_l_b")],
            axis=1)),
    }
    maps = []
    for core in range(NCORES):
        s, h = core // 2, core % 2
        m = dict(shared)
        oh = 1 - h
        m["xh"] = np.ascontiguousarray(xhp[s][:, 64 * h : 64 * h + XR, :XC])
        m["xh_ot"] = np.ascontiguousarray(c_hf[s][:, 64 * oh : 64 * oh + 64, :])
        m["xl"] = np.ascontiguousarray(xlp[s][:, 32 * h : 32 * h + XLR, :XLC])
        m["xl_ot"] = np.ascontiguousarray(c_lf[s][:, 32 * oh : 32 * oh + 32, :])
        m["sall"] = np.ascontiguousarray(np.stack([shp[s], slp[s]], axis=1))
        m["maskp"] = np.full((128, 1), float(h), np.float32)
        m["identp"] = np.eye(128, dtype=np.float32)
        maps.append(m)
    return maps


def _run(in_maps, trace=False, **kw):
    from concourse.bass_utils import run_bass_kernel_spmd

    if "nc" not in _CACHE:
        _CACHE["nc"] = _build_nc()
    return run_bass_kernel_spmd(
        _CACHE["nc"], in_maps, core_ids=list(range(NCORES)), trace=trace, **kw
    )


def kernel(**inputs):
    res = _run(_shard(inputs))
    hf = np.zeros((B, C, 128, 128), np.float32)
    lf = np.zeros((B, C, 64, 64), np.float32)
    for core in range(NCORES):
        s, h = core // 2, core % 2
        hf[s][:, 64 * h : 64 * h + OR_, :] = res.results[core]["out_hf"]
        lf[s][:, 32 * h : 32 * h + OLR, :] = res.results[core]["out_lf"]
    return hf, lf


# revision 37
# speedup vs baseline: 1.1711x; 1.0086x over previous
# AdaOctConv distributed Trainium2 kernel (8 NeuronCores, SPMD, no collectives).
#
# Sharding: 4 samples x 2 spatial halves = 8 cores. Each core computes a
# 64-row band of hf_out and a 32-row band of lf_out of one sample. Internal
# split halos come in via host-side overlapped shards; image-border halos are
# reflect-copies of computed interior rows, applied on device as mask-blended
# row/col fixups so all 8 cores share one SPMD graph.
#
# Math folding (per channel c, per sample):
#   instance_norm -> depthwise3x3(per-sample w) -> *w_point + bias  ==
#   depthwise3x3(raw x; ws*(wp*rsqrt(var+eps))) + beta, with beta folded into
#   the following dense conv's bias. The heavy work is six 3x3 dense convs,
#   each computed as 9 shift-matmuls (bf16 in, fp32 PSUM accumulation).
import sys

for _p in ("/opt/trn_rl_repo",):
    if _p not in sys.path:
        sys.path.append(_p)

import os

import numpy as np

B, C = 4, 256
EPS, SLOPE = 1e-5, 0.01
NCORES = 8

XR, XC = 72, 136      # x band (reflect-padded +-4)
ZR, ZC = 70, 134      # depthwise output band
HR, HC = 68, 132      # hf band
OR_, OC_ = 64, 128    # hf output band
XLR, XLC = 38, 70
ZLR, ZLC = 36, 68
LR, LC = 34, 66
OLR, OLC = 32, 64
UR, UC = 66, 130      # upsampled-lf band
UPT = 10              # rows per up tile (8 tiles of [8g, 8g+10))

_CACHE = {}


def _split_multi_waits(nc, mybir):
    """This walrus build accepts at most one sync-wait per instruction;
    hoist extra waits onto single-wait NOPs on the same engine."""
    for fn in nc.m.functions:
        for bb in fn.blocks:
            new = []
            for ins in bb.instructions:
                si = ins.sync_info
                if si and si.on_wait and len(si.on_wait) > 1:
                    waits = list(si.on_wait)
                    for w in waits[:-1]:
                        nop = mybir.InstNoOp(
                            name=f"I-ws-{nc.next_id()}", ins=[], outs=[]
                        )
                        nop.engine = ins.engine
                        nop.sync_info = mybir.SyncInfo(on_wait=[w], on_update=[])
                        new.append(nop)
                    ins.sync_info = mybir.SyncInfo(
                        on_wait=[waits[-1]], on_update=list(si.on_update or [])
                    )
                new.append(ins)
            bb.instructions[:] = new


def _build_nc():
    import contextlib

    import concourse.bass as bass
    import concourse.mybir as mybir
    import concourse.tile as tile

    f32 = mybir.dt.float32
    bf16 = mybir.dt.bfloat16
    AF = mybir.ActivationFunctionType
    OP = mybir.AluOpType
    AX = mybir.AxisListType

    STAGES = int(os.environ.get("ADAOCT_STAGES", "9"))
    SPLIT = os.environ.get("ADAOCT_SPLIT", "1") == "1"

    nc = bass.Bass()

    def par(name, shape, out=False):
        return nc.declare_dram_parameter(name, list(shape), f32, isOutput=out)

    xh = par("xh", (C, XR, XC))
    W1N = ("whpw", "whbw", "wlpw", "wlbw")
    BN = ("hsb", "hpb", "hbb", "ahb", "lsb", "lpb", "lbb", "alb")
    xh_ot = par("xh_ot", (C, 64, 128))
    xl = par("xl", (C, XLR, XLC))
    xl_ot = par("xl_ot", (C, 32, 64))
    identp = par("identp", (128, 128))
    maskp = par("maskp", (128, 1))
    wsets = {
        n: par(n, (C, 9, C))
        for n in ("whsw", "wlsw", "wadal", "wadah", "wh2h", "wl2h", "wl2l", "wh2l")
    }
    w1all = par("w1all", (C, 4 * C))  # whpw|whbw|wlpw|wlbw along cols
    ball = par("ball", (C, 8))  # hsb hpb hbb ahb lsb lpb lbb alb
    sall = par("sall", (C, 2, 5, 5))  # sh | sl
    out_hf = par("out_hf", (C, OR_, OC_), out=True)
    out_lf = par("out_lf", (C, OLR, OLC), out=True)

    TAPS = [(dy, dx) for dy in range(3) for dx in range(3)]

    class _StopBuild(Exception):
        pass

    with tile.TileContext(nc) as tc:
        es = contextlib.ExitStack()
        with es, contextlib.suppress(_StopBuild):
            persist = es.enter_context(tc.tile_pool(name="persist", bufs=1))
            small = es.enter_context(tc.tile_pool(name="small", bufs=1))
            wpool = es.enter_context(tc.tile_pool(name="wpool", bufs=4))
            wstg = es.enter_context(tc.tile_pool(name="wstg", bufs=2))
            psmall = es.enter_context(tc.tile_pool(name="psmall", bufs=2, space="PSUM"))
            mmp = es.enter_context(tc.tile_pool(name="mmp", bufs=6, space="PSUM"))
            accp = es.enter_context(tc.tile_pool(name="accp", bufs=1))
            z2p = es.enter_context(tc.tile_pool(name="z2p", bufs=4))
            upp = es.enter_context(tc.tile_pool(name="upp", bufs=2))
            outp = es.enter_context(tc.tile_pool(name="outp", bufs=2))
            fixp = es.enter_context(tc.tile_pool(name="fixp", bufs=1))

            # ---------- persistent tensors ----------
            xh_sb = [persist.tile([128, XR, XC], bf16, tag=f"xh{i}") for i in range(2)]
            xl_sb = [persist.tile([128, XLR, XLC], bf16, tag=f"xl{i}") for i in range(2)]
            hf_sb = [persist.tile([128, HR, HC], bf16, tag=f"hf{i}") for i in range(2)]
            lf_sb = [persist.tile([128, LR, LC], bf16, tag=f"lf{i}") for i in range(2)]
            avg_sb = [persist.tile([128, LR, LC], bf16, tag=f"av{i}") for i in range(2)]

            eps_t = small.tile([128, 1], f32, tag="eps")
            nc.vector.memset(eps_t, EPS)
            mask_t = small.tile([128, 1], f32, tag="mask")
            nc.sync.dma_start(out=mask_t, in_=maskp[:, :])
            imask_t = small.tile([128, 1], f32, tag="imask")
            nc.scalar.activation(
                out=imask_t, in_=mask_t, func=AF.Copy, bias=1.0, scale=-1.0
            )

            identf = small.tile([128, 128], f32, tag="identf", name="identf")
            nc.sync.dma_start(out=identf, in_=identp[:, :])
            ident_b = small.tile([128, 128], bf16, tag="identb", name="identb")
            nc.scalar.copy(out=ident_b, in_=identf)

            def blend(cur, ref, sel, cols):
                """cur = cur + sel*(ref - cur) over a [128, 1, cols] slice."""
                d = fixp.tile([128, 1, 144], f32, tag="fixd")
                dd = d[:, :, :cols]
                nc.vector.tensor_sub(out=dd, in0=ref, in1=cur)
                nc.vector.scalar_tensor_tensor(
                    out=cur, in0=dd, scalar=sel, in1=cur, op0=OP.mult, op1=OP.add
                )

            bias_sb = {}
            for i in range(2):
                bt = small.tile([128, 8], f32, tag=f"ball{i}", name=f"ball{i}")
                nc.sync.dma_start(out=bt, in_=ball[i * 128 : (i + 1) * 128, :])
                for j, n in enumerate(BN):
                    bias_sb[(n, i)] = bt[:, j : j + 1]

            w1_sb = {}
            for i in range(2):
                stg = wstg.tile([128, 1152], f32, tag="wstg", name="w1stg")
                nc.sync.dma_start(
                    out=stg[:, : 4 * C].rearrange("c (k o) -> c k o", k=4),
                    in_=w1all[i * 128 : (i + 1) * 128, :].rearrange(
                        "c (k o) -> c k o", k=4),
                )
                t = small.tile([128, 4, C], bf16, tag=f"w1_{i}", name=f"w1_{i}")
                nc.scalar.copy(out=t, in_=stg[:, : 4 * C].rearrange(
                    "c (k o) -> c k o", k=4))
                for j, n in enumerate(W1N):
                    w1_sb[(n, i)] = t[:, j, :]

            sbuf_s = {}
            for i in range(2):
                tf = small.tile([128, 2, 5, 5], f32, tag=f"sf{i}", name=f"sf{i}")
                nc.sync.dma_start(out=tf, in_=sall[i * 128 : (i + 1) * 128, :, :, :])
                tb = small.tile([128, 2, 5, 5], bf16, tag=f"sb{i}", name=f"sb{i}")
                nc.scalar.copy(out=tb, in_=tf)
                for j, nm in enumerate(("sh", "sl")):
                    sbuf_s[(nm, i, "f")] = tf[:, j]
                    sbuf_s[(nm, i, "b")] = tb[:, j]

            # ---------- weight set loading ----------
            wcache = {}

            def wset(name):
                if name in wcache:
                    return wcache[name]
                p = wsets[name]
                tiles = []
                for i in range(2):
                    t = wpool.tile([128, 9, 2, 128], bf16, tag="wset")
                    flat = t.rearrange("c t o q -> c (t o q)")
                    src = p[i * 128 : (i + 1) * 128, :, :].rearrange("c t o -> c (t o)")
                    for j in range(2):
                        stg = wstg.tile([128, 1152], f32, tag="wstg")
                        nc.sync.dma_start(
                            out=stg, in_=src[:, j * 1152 : (j + 1) * 1152]
                        )
                        nc.scalar.copy(out=flat[:, j * 1152 : (j + 1) * 1152], in_=stg)
                    tiles.append(t)
                wcache[name] = tiles
                return tiles

            # ---------- instance-norm stats ----------
            # mean/var over [own band interior (bf16, in SBUF)] + [other
            # half's rows, streamed f32]. Sums via ScalarE accum_out.
            def stats_stream(other, hw, bpool):
                npc = (hw // 2) // 1024
                bn_t = []
                for i in range(2):
                    nm = f"{other.name}{i}"
                    bnst = small.tile([128, 2 * npc, 6], f32, tag=f"bn_{nm}",
                                      name=f"bn_{nm}")
                    flat = other[i * 128 : (i + 1) * 128, :, :].rearrange(
                        "c h w -> c (h w)"
                    )
                    for k in range(npc):
                        pc = bpool.tile([128, 1024], f32, tag="xstg2", name="xstg2")
                        nc.gpsimd.dma_start(
                            out=pc, in_=flat[:, k * 1024 : (k + 1) * 1024]
                        )
                        for j in range(2):
                            nc.vector.bn_stats(
                                out=bnst[:, 2 * k + j, :],
                                in_=pc[:, j * 512 : (j + 1) * 512],
                            )
                    bn_t.append(bnst)
                return bn_t

            def stats(bn_t, band_sb, bint, hw, bpool):
                r0, r1, c0, c1 = bint
                half = hw // 2
                m_t, inv_t = [], []
                for i in range(2):
                    nm = bn_t[i].tensor.name
                    sums = small.tile([128, 5, 2], f32, tag=f"s5_{nm}",
                                      name=f"s5_{nm}")
                    mvs = small.tile([128, 2], f32, tag=f"mvs_{nm}",
                                     name=f"mvs_{nm}")
                    nc.vector.bn_aggr(out=mvs, in_=bn_t[i])
                    # stream half back to (sum, sumsq) space
                    nc.vector.tensor_scalar_mul(out=sums[:, 4, 0:1],
                                                in0=mvs[:, 0:1], scalar1=float(half))
                    msq = small.tile([128, 1], f32, tag=f"msq_{nm}",
                                     name=f"msq_{nm}")
                    nc.vector.tensor_mul(out=msq, in0=mvs[:, 0:1], in1=mvs[:, 0:1])
                    nc.vector.tensor_add(out=msq, in0=msq, in1=mvs[:, 1:2])
                    nc.vector.tensor_scalar_mul(out=sums[:, 4, 1:2], in0=msq,
                                                scalar1=float(half))
                    rows = r1 - r0
                    q = rows // 4
                    for j in range(4):
                        seg = band_sb[i][:, r0 + j * q : r0 + (j + 1) * q, c0:c1]
                        scr = bpool.tile([128, 2048], bf16, tag="scr", name="scr",
                                         bufs=1)
                        sg = scr[:, : seg.free_size()]
                        nc.scalar.activation(out=sg, in_=seg, func=AF.Copy,
                                             accum_out=sums[:, j, 0:1])
                        nc.scalar.activation(out=sg, in_=seg, func=AF.Square,
                                             accum_out=sums[:, j, 1:2])
                    mv = small.tile([128, 2], f32, tag=f"mv_{nm}", name=f"mv_{nm}")
                    nc.vector.tensor_reduce(
                        out=mv, in_=sums.rearrange("c n k -> c k n"),
                        axis=AX.X, op=OP.add)
                    m = small.tile([128, 1], f32, tag=f"m_{nm}", name=f"m_{nm}")
                    nc.vector.tensor_scalar_mul(out=m, in0=mv[:, 0:1],
                                                scalar1=1.0 / hw)
                    var = small.tile([128, 1], f32, tag=f"v_{nm}", name=f"v_{nm}")
                    nc.vector.tensor_scalar_mul(out=var, in0=mv[:, 1:2],
                                                scalar1=1.0 / hw)
                    msqt = small.tile([128, 1], f32, tag=f"mq_{nm}", name=f"mq_{nm}")
                    nc.vector.tensor_mul(out=msqt, in0=m, in1=m)
                    nc.vector.tensor_sub(out=var, in0=var, in1=msqt)
                    inv = small.tile([128, 1], f32, tag=f"i_{nm}", name=f"i_{nm}")
                    nc.scalar.activation(
                        out=inv, in_=var, func=AF.Sqrt, bias=eps_t, scale=1.0
                    )
                    nc.vector.reciprocal(out=inv, in_=inv)
                    m_t.append(m)
                    inv_t.append(inv)
                return m_t, inv_t

            def load_band(param, dst, rows, cols, bpool, rchunk):
                for i in range(2):
                    src = param[i * 128 : (i + 1) * 128, :, :]
                    for r0 in range(0, rows, rchunk):
                        r1 = min(r0 + rchunk, rows)
                        stg = bpool.tile([128, 3264], f32, tag="xstg")
                        s3 = stg[:, : (r1 - r0) * cols].rearrange(
                            "c (r w) -> c r w", w=cols
                        )
                        nc.gpsimd.dma_start(out=s3, in_=src[:, r0:r1, :])
                        nc.scalar.copy(out=dst[i][:, r0:r1, :], in_=s3)

            bpool = es.enter_context(tc.tile_pool(name="bpool", bufs=2))
            wset("wlsw")
            wset("wadal")
            sums_l = stats_stream(xl_ot, 64 * 64, bpool)
            load_band(xl, xl_sb, XLR, XLC, bpool, 14)
            m_l, inv_l = stats(sums_l, xl_sb, (3, 35, 3, 67), 64 * 64, bpool)

            # ---------- kernel_predict + folds ----------
            # ws2 = ws*wp (stats-free; inv is folded into the dense-conv
            # stationary later). beta = b - (wp*inv)*m*S.
            def kp_A(skey, wsw_name, w1pw, w1bw, bsb, bpb, bbb, tag, want_diag=False):
                wsw = wset(wsw_name)
                s_f = [sbuf_s[(skey, i, "f")] for i in range(2)]
                s_b = [sbuf_s[(skey, i, "b")] for i in range(2)]
                pooled_b = []
                for i in range(2):
                    pf = small.tile([128, 1], f32, tag=f"poo{tag}{i}", name=f"poo{tag}{i}")
                    nc.vector.reduce_sum(out=pf, in_=s_f[i][:, 1:4, 1:4], axis=AX.XY)
                    pb = small.tile([128, 1], bf16, tag=f"poob{tag}{i}", name=f"poob{tag}{i}")
                    nc.scalar.copy(out=pb, in_=pf)
                    pooled_b.append(pb)

                ws2, wp_t, bv_t, S_t, diag = [], [], [], [], []
                for oc in range(2):
                    ps = psmall.tile([128, 9], f32, tag="ps9", name="ps9")
                    ps3 = ps.rearrange("c (a b) -> c a b", a=3)
                    n = 0
                    for t, (dy, dx) in enumerate(TAPS):
                        for ic in range(2):
                            nc.tensor.matmul(
                                ps3,
                                wsw[ic][:, t, oc, :],
                                s_b[ic][:, dy : dy + 3, dx : dx + 3],
                                start=(n == 0),
                                stop=(n == 17),
                            )
                            n += 1
                    wf = small.tile([128, 9], f32, tag=f"ws{tag}{oc}", name=f"ws{tag}{oc}")
                    nc.scalar.activation(out=wf, in_=ps, func=AF.Identity,
                                         bias=bias_sb[(bsb, oc)], scale=1.0)

                    ps1 = psmall.tile([128, 9], f32, tag="ps9", name="ps9")
                    for ic in range(2):
                        nc.tensor.matmul(
                            ps1[:, 0:1],
                            w1_sb[(w1pw, ic)][:, oc * 128 : (oc + 1) * 128],
                            pooled_b[ic],
                            start=(ic == 0),
                            stop=(ic == 1),
                        )
                    wp = small.tile([128, 1], f32, tag=f"wp{tag}{oc}", name=f"wp{tag}{oc}")
                    nc.scalar.activation(out=wp, in_=ps1[:, 0:1], func=AF.Identity,
                                         bias=bias_sb[(bpb, oc)], scale=1.0)
                    wp_t.append(wp)

                    ps2 = psmall.tile([128, 9], f32, tag="ps9", name="ps9")
                    for ic in range(2):
                        nc.tensor.matmul(
                            ps2[:, 0:1],
                            w1_sb[(w1bw, ic)][:, oc * 128 : (oc + 1) * 128],
                            pooled_b[ic],
                            start=(ic == 0),
                            stop=(ic == 1),
                        )
                    bv = small.tile([128, 1], f32, tag=f"bv{tag}{oc}", name=f"bv{tag}{oc}")
                    nc.scalar.activation(out=bv, in_=ps2[:, 0:1], func=AF.Identity,
                                         bias=bias_sb[(bbb, oc)], scale=1.0)
                    bv_t.append(bv)

                    w2 = small.tile([128, 9], f32, tag=f"w2{tag}{oc}", name=f"w2{tag}{oc}")
                    nc.scalar.activation(out=w2, in_=wf, func=AF.Identity,
                                         bias=0.0, scale=wp)
                    ws2.append(w2)
                    s_s = small.tile([128, 1], f32, tag=f"S{tag}{oc}", name=f"S{tag}{oc}")
                    nc.vector.reduce_sum(out=s_s, in_=wf, axis=AX.X)
                    S_t.append(s_s)
                    if want_diag:
                        dg = small.tile([128, 9, 128], bf16, tag=f"dg{tag}{oc}", name=f"dg{tag}{oc}")
                        for t in range(9):
                            nc.scalar.activation(
                                out=dg[:, t, :], in_=ident_b, func=AF.Identity,
                                bias=0.0, scale=w2[:, t : t + 1])
                        diag.append(dg)
                return ws2, diag, wp_t, bv_t, S_t

            def kp_B(wada_name, bada, wp_t, bv_t, S_t, m_t, inv_t, tag):
                """bias2 from unscaled wada, then scale wada in place by
                inv[cin] (per-partition)."""
                wada = wset(wada_name)
                beta_b = []
                for oc in range(2):
                    tmp = small.tile([128, 1], f32, tag=f"t1{tag}{oc}", name=f"t1{tag}{oc}")
                    nc.scalar.activation(out=tmp, in_=wp_t[oc], func=AF.Identity,
                                         bias=0.0, scale=inv_t[oc])
                    nc.scalar.activation(out=tmp, in_=tmp, func=AF.Identity,
                                         bias=0.0, scale=m_t[oc])
                    nc.scalar.activation(out=tmp, in_=tmp, func=AF.Identity,
                                         bias=0.0, scale=S_t[oc])
                    nc.scalar.mul(out=tmp, in_=tmp, mul=-1.0)
                    bb_ = small.tile([128, 1], bf16, tag=f"beb{tag}{oc}", name=f"beb{tag}{oc}")
                    nc.scalar.activation(out=bb_, in_=bv_t[oc], func=AF.Identity,
                                         bias=tmp, scale=1.0)
                    beta_b.append(bb_)
                bias2 = []
                for oc in range(2):
                    ps = psmall.tile([128, 9], f32, tag="ps9", name="ps9")
                    n = 0
                    for t in range(9):
                        for ic in range(2):
                            nc.tensor.matmul(
                                ps[:, 0:1],
                                wada[ic][:, t, oc, :],
                                beta_b[ic],
                                start=(n == 0),
                                stop=(n == 17),
                            )
                            n += 1
                    b2 = small.tile([128, 1], f32, tag=f"b2{tag}{oc}", name=f"b2{tag}{oc}")
                    nc.scalar.activation(out=b2, in_=ps[:, 0:1], func=AF.Identity,
                                         bias=bias_sb[(bada, oc)], scale=1.0)
                    bias2.append(b2)
                for ic in range(2):
                    wf_ = wada[ic].rearrange("c t o q -> c (t o q)")
                    nc.scalar.activation(out=wf_, in_=wf_, func=AF.Identity,
                                         bias=0.0, scale=inv_t[ic])
                return bias2, wada

            # ---------- depthwise (9-tap MAC on the Vector engine) ----------
            def dw_tile(x_sb, ws2, z0, zrows, W, tag, fix):
                """z[c, q, w] = sum_t ws2[c,t] * x[c, q+dy, w+dx]; cols
                [0, W-2) valid. Returns [128, zrows, W] bf16 views."""
                topq, tops, botq, bots, ctop, cts, cbot, cbs = fix
                zc = W - 2
                out = []
                for ic in range(2):
                    acc = accp.tile([128, 14, 134], bf16, tag="acc", name="acc")
                    a = acc[:, :zrows, :zc]
                    zt = z2p.tile([128, 14, zc], bf16, tag=f"z2{tag}", name=f"z2{tag}")
                    z = zt[:, :zrows, :]
                    zv = z
                    for t, (dy, dx) in enumerate(TAPS):
                        in0 = x_sb[ic][:, z0 + dy : z0 + dy + zrows, dx : dx + zc]
                        sc = ws2[ic][:, t : t + 1]
                        if t == 0:
                            nc.vector.tensor_scalar_mul(out=a, in0=in0, scalar1=sc)
                        elif t < 8:
                            nc.vector.scalar_tensor_tensor(
                                out=a, in0=in0, scalar=sc, in1=a,
                                op0=OP.mult, op1=OP.add,
                            )
                        else:
                            nc.vector.scalar_tensor_tensor(
                                out=zv, in0=in0, scalar=sc, in1=a,
                                op0=OP.mult, op1=OP.add,
                            )
                    if z0 <= topq < z0 + zrows:
                        blend(z[:, topq - z0 : topq - z0 + 1, :zc],
                              z[:, tops - z0 : tops - z0 + 1, :zc], imask_t, zc)
                    if z0 <= botq < z0 + zrows:
                        blend(z[:, botq - z0 : botq - z0 + 1, :zc],
                              z[:, bots - z0 : bots - z0 + 1, :zc], mask_t, zc)
                    nc.vector.tensor_copy(
                        out=z[:, :, ctop : ctop + 1], in_=z[:, :, cts : cts + 1]
                    )
                    nc.vector.tensor_copy(
                        out=z[:, :, cbot : cbot + 1], in_=z[:, :, cbs : cbs + 1]
                    )
                    out.append(z)
                return out

            def dw_tile_pe(x_sb, diag, z0, zrows, W, tag, fix):
                """Same contract as dw_tile, but computed on the TensorEngine
                as 9 accumulated diag-matmuls over flat 512-col windows."""
                topq, tops, botq, bots, ctop, cts, cbot, cbs = fix
                L = zrows * W - 2
                out = []
                for ic in range(2):
                    zt = z2p.tile([128, 14, W], bf16, tag=f"z2{tag}", name=f"z2p{tag}")
                    zflat = zt.rearrange("c r w -> c (r w)")
                    xflat = x_sb[ic].rearrange("c r w -> c (r w)")
                    for off in range(0, L, 512):
                        n = min(512, L - off)
                        ps = mmp.tile([128, 512], f32, tag="mm", name="zps")
                        for t, (dy, dx) in enumerate(TAPS):
                            base = (z0 + dy) * W + dx + off
                            nc.tensor.matmul(
                                ps[:, :n],
                                diag[ic][:, t, :],
                                xflat[:, base : base + n],
                                start=(t == 0),
                                stop=(t == 8),
                            )
                        nc.scalar.copy(out=zflat[:, off : off + n], in_=ps[:, :n])
                    z = zt[:, :zrows, :]
                    if z0 <= topq < z0 + zrows:
                        blend(z[:, topq - z0 : topq - z0 + 1, : W - 2],
                              z[:, tops - z0 : tops - z0 + 1, : W - 2], imask_t, W - 2)
                    if z0 <= botq < z0 + zrows:
                        blend(z[:, botq - z0 : botq - z0 + 1, : W - 2],
                              z[:, bots - z0 : bots - z0 + 1, : W - 2], mask_t, W - 2)
                    nc.vector.tensor_copy(
                        out=z[:, :, ctop : ctop + 1], in_=z[:, :, cts : cts + 1]
                    )
                    nc.vector.tensor_copy(
                        out=z[:, :, cbot : cbot + 1], in_=z[:, :, cbs : cbs + 1]
                    )
                    out.append(z)
                return out

            def conv_block(psum, srcs, oc):
                n = 0
                total = 18 * len(srcs)
                for tiles, wt, r0, c0, rows, cols in srcs:
                    for t, (dy, dx) in enumerate(TAPS):
                        for ic in range(2):
                            nc.tensor.matmul(
                                psum,
                                wt[ic][:, t, oc, :],
                                tiles[ic][:, r0 + dy : r0 + dy + rows,
                                          c0 + dx : c0 + dx + cols],
                                start=(n == 0),
                                stop=(n == total - 1),
                            )
                            n += 1

            def band_fixups(sb, topq, tops, botq, bots, ctop, cts, cbot, cbs, cols):
                for ic in range(2):
                    blend(sb[ic][:, topq : topq + 1, :],
                          sb[ic][:, tops : tops + 1, :], imask_t, cols)
                    blend(sb[ic][:, botq : botq + 1, :],
                          sb[ic][:, bots : bots + 1, :], mask_t, cols)
                    nc.vector.tensor_copy(
                        out=sb[ic][:, :, ctop : ctop + 1],
                        in_=sb[ic][:, :, cts : cts + 1],
                    )
                    nc.vector.tensor_copy(
                        out=sb[ic][:, :, cbot : cbot + 1],
                        in_=sb[ic][:, :, cbs : cbs + 1],
                    )

            # ================= LF branch =================
            if STAGES < 2:
                dbg = persist.tile([128, 2048], f32, tag="dbg", name="dbg")
                nc.vector.memset(dbg, 0.0)
                d3 = dbg.rearrange("c (r w) -> c r w", w=OC_)
                for oc in range(2):
                    for r0 in range(0, OR_, 16):
                        nc.sync.dma_start(
                            out=out_hf[oc * 128 : (oc + 1) * 128, r0 : r0 + 16, :],
                            in_=d3)
                d4 = dbg[:, : 16 * OLC].rearrange("c (r w) -> c r w", w=OLC)
                for oc in range(2):
                    for r0 in range(0, OLR, 16):
                        nc.sync.dma_start(
                            out=out_lf[oc * 128 : (oc + 1) * 128, r0 : r0 + 16, :],
                            in_=d4)
                raise _StopBuild()
            ws2_l, diag_l, wp_l, bv_l, S_l = kp_A(
                "sl", "wlsw", "wlpw", "wlbw", "lsb", "lpb", "lbb", "l"
            )
            bias2_l, wadal = kp_B("wadal", "alb", wp_l, bv_l, S_l, m_l, inv_l, "l")

            # hf stats stream first (inv_h gates the ada_h stationary),
            # then the hf band
            sums_h = stats_stream(xh_ot, 128 * 128, bpool)
            load_band(xh, xh_sb, XR, XC, bpool, 7)
            ws2_h, diag_h, wp_h, bv_h, S_h = kp_A(
                "sh", "whsw", "whpw", "whbw", "hsb", "hpb", "hbb", "h",
                want_diag=True,
            )

            if STAGES < 3:
                raise _StopBuild()
            zl_fix = (1, 3, 34, 32, 1, 3, 66, 64)
            for t0 in range(3):
                z0 = 12 * t0
                zt = dw_tile(xl_sb, ws2_l, z0, min(14, ZLR - z0), XLC, "l", zl_fix)
                if STAGES < 4:
                    continue
                for b in range(2 * t0, min(2 * t0 + 2, 6)):
                    r0 = 6 * b
                    rb = min(6, LR - r0)
                    q0 = r0 - 12 * t0
                    for oc in range(2):
                        ps = mmp.tile([128, 6, LC], f32, tag="mm", name="mm")
                        p = ps[:, :rb, :]
                        conv_block(p, [(zt, wadal, q0, 0, rb, LC)], oc)
                        nc.scalar.activation(
                            out=lf_sb[oc][:, r0 : r0 + rb, :], in_=p,
                            func=AF.Lrelu, bias=bias2_l[oc], scale=1.0, alpha=SLOPE,
                        )
            if STAGES < 4:
                raise _StopBuild()
            band_fixups(lf_sb, 0, 2, 33, 31, 0, 2, 65, 63, LC)

            # ================= HF branch =================
            m_h, inv_h = stats(sums_h, xh_sb, (4, 68, 4, 132), 128 * 128, bpool)
            bias2_h, wadah = kp_B("wadah", "ahb", wp_h, bv_h, S_h, m_h, inv_h, "h")

            zh_fix = (2, 4, 67, 65, 2, 4, 131, 129)
            for t0 in range(6):
                z0 = 12 * t0
                zrows = min(14, ZR - z0)
                if t0 in (0, 2, 4):
                    zt = dw_tile_pe(xh_sb, diag_h, z0, zrows, XC, "h", zh_fix)
                else:
                    zt = dw_tile(xh_sb, ws2_h, z0, zrows, XC, "h", zh_fix)
                for b in range(4 * t0, min(4 * t0 + 4, 23)):
                    r0 = 3 * b
                    rb = min(3, HR - r0)
                    q0 = r0 - 12 * t0
                    for oc in range(2):
                        ps = mmp.tile([128, 3, HC], f32, tag="mm", name="mm")
                        p = ps[:, :rb, :]
                        conv_block(p, [(zt, wadah, q0, 0, rb, HC)], oc)
                        nc.scalar.activation(
                            out=hf_sb[oc][:, r0 : r0 + rb, :], in_=p,
                            func=AF.Lrelu, bias=bias2_h[oc], scale=1.0, alpha=SLOPE,
                        )
            band_fixups(hf_sb, 1, 3, 66, 64, 1, 3, 130, 128, HC)

            # ================= cross-frequency fusion =================
            if STAGES < 5:
                raise _StopBuild()
            wh2h = wset("wh2h")
            wl2h = wset("wl2h")

            # special up rows: u=0 -> lf[1]+mask*(lf[0]-lf[1]);
            #                  u=65 -> lf[33]+mask*(lf[32]-lf[33])
            sprow = {}
            for key, ja, jb in (("r0", 1, 0), ("r65", 33, 32)):
                rows = []
                for ic in range(2):
                    d = small.tile([128, 1, LC], f32, tag=f"upd{key}{ic}")
                    nc.vector.tensor_sub(
                        out=d, in0=lf_sb[ic][:, jb : jb + 1, :],
                        in1=lf_sb[ic][:, ja : ja + 1, :],
                    )
                    r = small.tile([128, 1, LC], bf16, tag=f"upr{key}{ic}")
                    nc.vector.scalar_tensor_tensor(
                        out=r, in0=d, scalar=mask_t,
                        in1=lf_sb[ic][:, ja : ja + 1, :],
                        op0=OP.mult, op1=OP.add,
                    )
                    rows.append(r)
                sprow[key] = rows

            def up_cols(dst_rows, src_rows):
                # dst [128, n, 130] <- src [128, n, 64] column-doubling w/ edges
                nc.vector.tensor_copy(out=dst_rows[:, :, 1:129:2], in_=src_rows)
                nc.vector.tensor_copy(out=dst_rows[:, :, 2:130:2], in_=src_rows)
                nc.vector.tensor_copy(
                    out=dst_rows[:, :, 0:1], in_=src_rows[:, :, 0:1]
                )
                nc.vector.tensor_copy(
                    out=dst_rows[:, :, 129:130], in_=src_rows[:, :, 63:64]
                )

            def build_up_tile(g):
                tiles = []
                u0 = 8 * g
                for ic in range(2):
                    ut = upp.tile([128, UPT, UC], bf16, tag="up")
                    ev = [i for i in range(0, UPT, 2) if not (g == 0 and i == 0)]
                    od = [i for i in range(1, UPT, 2) if not (g == 7 and i == 9)]
                    for phase in (ev, od):
                        i0, cnt = phase[0], len(phase)
                        j0 = (u0 + i0 - 1) // 2 + 1
                        dst = ut[:, i0 : i0 + 2 * cnt - 1 : 2, :]
                        src = lf_sb[ic][:, j0 : j0 + cnt, 1:65]
                        up_cols(dst, src)
                    if g == 0:
                        up_cols(ut[:, 0:1, :], sprow["r0"][ic][:, :, 1:65])
                    if g == 7:
                        up_cols(ut[:, 9:10, :], sprow["r65"][ic][:, :, 1:65])
                    tiles.append(ut)
                return tiles

            if STAGES < 6:
                for g in range(8):
                    build_up_tile(g)
                raise _StopBuild()
            up_tiles = {}
            for r in range(16):
                g = r // 2
                if g not in up_tiles:
                    up_tiles[g] = build_up_tile(g)
                u_local = 4 * r - 8 * g
                for oc in range(2):
                    ps = mmp.tile([128, 4, OC_], f32, tag="mm")
                    conv_block(
                        ps,
                        [
                            (hf_sb, wh2h, 4 * r + 1, 1, 4, OC_),
                            (up_tiles[g], wl2h, u_local, 0, 4, OC_),
                        ],
                        oc,
                    )
                    stg = outp.tile([128, 4, OC_], f32, tag="ostg")
                    nc.scalar.activation(
                        out=stg, in_=ps, func=AF.Lrelu, bias=0.0, scale=1.0,
                        alpha=SLOPE,
                    )
                    nc.sync.dma_start(
                        out=out_hf[oc * 128 : (oc + 1) * 128, 4 * r : 4 * r + 4, :],
                        in_=stg,
                    )

            # avgpool of hf (0.25 folded into h2l weights host-side)
            for ic in range(2):
                h4 = hf_sb[ic].rearrange("c (r p) (w q) -> c r p w q", p=2, q=2)
                s1 = accp.tile([128, LR, LC], bf16, tag="avt")
                nc.vector.tensor_add(
                    out=s1, in0=h4[:, :, 0, :, 0], in1=h4[:, :, 0, :, 1]
                )
                s2 = accp.tile([128, LR, LC], bf16, tag="avt")
                nc.vector.tensor_add(
                    out=s2, in0=h4[:, :, 1, :, 0], in1=h4[:, :, 1, :, 1]
                )
                nc.vector.tensor_add(out=avg_sb[ic], in0=s1, in1=s2)
            band_fixups(avg_sb, 0, 2, 33, 31, 0, 2, 65, 63, LC)

            wl2l = wset("wl2l")
            wh2l = wset("wh2l")
            for b in range(6):
                r0 = 6 * b
                rb = min(6, OLR - r0)
                for oc in range(2):
                    ps = mmp.tile([128, 6, OLC], f32, tag="mm")
                    p = ps[:, :rb, :]
                    conv_block(
                        p,
                        [
                            (lf_sb, wl2l, r0, 0, rb, OLC),
                            (avg_sb, wh2l, r0, 0, rb, OLC),
                        ],
                        oc,
                    )
                    stg = outp.tile([128, 6, OLC], f32, tag="ostg2")
                    sg = stg.rearrange("c r w -> c (r w)")[:, : rb * OLC].rearrange(
                        "c (r w) -> c r w", w=OLC)
                    nc.scalar.activation(
                        out=sg, in_=p, func=AF.Lrelu, bias=0.0, scale=1.0,
                        alpha=SLOPE,
                    )
                    nc.sync.dma_start(
                        out=out_lf[oc * 128 : (oc + 1) * 128, r0 : r0 + rb, :],
                        in_=sg,
                    )

    if SPLIT:
        _split_multi_waits(nc, mybir)
    return nc


def _shard(inputs):
    f = lambda k: np.ascontiguousarray(np.asarray(inputs[k], dtype=np.float32))
    c_hf, c_lf, s_hf, s_lf = f("c_hf"), f("c_lf"), f("s_hf"), f("s_lf")
    xhp = np.pad(c_hf, ((0, 0), (0, 0), (4, 4), (4, 4)), mode="reflect")
    xlp = np.pad(c_lf, ((0, 0), (0, 0), (3, 3), (3, 3)), mode="reflect")
    shp = np.pad(s_hf, ((0, 0), (0, 0), (1, 1), (1, 1)), mode="reflect")
    slp = np.pad(s_lf, ((0, 0), (0, 0), (1, 1), (1, 1)), mode="reflect")

    w9 = lambda k, s=1.0: np.ascontiguousarray(
        f(k).reshape(C, C, 9).transpose(1, 2, 0) * s
    )  # [cin, tap, cout]
    wT = lambda k, s=1.0: np.ascontiguousarray(f(k).reshape(C, C).T * s)
    col = lambda k: np.ascontiguousarray(f(k).reshape(C, 1))

    shared = {
        "whsw": w9("h_sw"), "wlsw": w9("l_sw"),
        "wadah": w9("ada_h_w"), "wadal": w9("ada_l_w"),
        "wh2h": w9("h2h"), "wl2h": w9("l2h"),
        "wl2l": w9("l2l"), "wh2l": w9("h2l", 0.25),
        "w1all": np.ascontiguousarray(np.concatenate(
            [wT("h_pw", 1 / 9.0), wT("h_bw", 1 / 9.0),
             wT("l_pw", 1 / 9.0), wT("l_bw", 1 / 9.0)], axis=1)),
        "ball": np.ascontiguousarray(np.stack(
            [f(k).reshape(C) for k in ("h_sb", "h_pb", "h_bb", "ada_h_b",
                                       "l_sb", "l_pb", "l_bb", "ada_l_b")],
            axis=1)),
    }
    maps = []
    for core in range(NCORES):
        s, h = core // 2, core % 2
        m = dict(shared)
        oh = 1 - h
        m["xh"] = np.ascontiguousarray(xhp[s][:, 64 * h : 64 * h + XR, :XC])
        m["xh_ot"] = np.ascontiguousarray(c_hf[s][:, 64 * oh : 64 * oh + 64, :])
        m["xl"] = np.ascontiguousarray(xlp[s][:, 32 * h : 32 * h + XLR, :XLC])
        m["xl_ot"] = np.ascontiguousarray(c_lf[s][:, 32 * oh : 32 * oh + 32, :])
        m["sall"] = np.ascontiguousarray(np.stack([shp[s], slp[s]], axis=1))
        m["maskp"] = np.full((128, 1), float(h), np.float32)
        m["identp"] = np.eye(128, dtype=np.float32)
        maps.append(m)
    return maps


def _run(in_maps, trace=False, **kw):
    from concourse.bass_utils import run_bass_kernel_spmd

    if "nc" not in _CACHE:
        _CACHE["nc"] = _build_nc()
    return run_bass_kernel_spmd(
        _CACHE["nc"], in_maps, core_ids=list(range(NCORES)), trace=trace, **kw
    )


def kernel(**inputs):
    res = _run(_shard(inputs))
    hf = np.zeros((B, C, 128, 128), np.float32)
    lf = np.zeros((B, C, 64, 64), np.float32)
    for core in range(NCORES):
        s, h = core // 2, core % 2
        hf[s][:, 64 * h : 64 * h + OR_, :] = res.results[core]["out_hf"]
        lf[s][:, 32 * h : 32 * h + OLR, :] = res.results[core]["out_lf"]
    return hf, lf
